# revision 1
# baseline (speedup 1.0000x reference)
"""CrossTemporalAttention2 Trainium2 kernel.

Sharding: 8 cores = 2 batches x 4 query-chunks of 1024 rows.
Each core: full conv+LN+KV pipeline for its batch (duplicated across the 4
cores of the batch group), attention + proj for its 1024 query rows.

Everything on-chip runs in "T space" (channels on partitions):
  x1T/x2T [C, N] -> conv (as matmul over (c,kh,kw) patches) -> xrT [C, M]
  -> LayerNorm via ones-matmul column stats + broadcast-matmul
  -> k2T [C, M], v1 [M, C] (natural, for AV stationary), qT [C, n-chunk]
  -> scoresT [M, n] per head (row-packed 4 heads/PE pass)
  -> exp on ACT (no max subtraction; values are small by construction)
  -> U^T = v1^T @ E^T (col-packed 4 heads), denom via DVE/GPSIMD tree +
     select-matmul, recip, broadcast-matmul, divide
  -> projT -> yT [C, n-chunk] -> DRAM (host transposes back)

Host prep: transposes x1/x2 per batch, folds ln_g into kv weights, folds the
softmax scale into q_w, reshapes sr_w into [kh, kw, cin, cout].
"""

import numpy as np

B, N, C = 2, 4096, 256
H, Dh = 8, 32
M = 1024          # (64/2) * (64/2)
NCH = 1024        # query rows per core
SCALE = Dh ** -0.5
EPS = 1e-5

_prog_cache = {}


def _build_program():
    import concourse.bass as bass
    import concourse.bacc as bacc
    import concourse.tile as tile
    from concourse import mybir
    from contextlib import ExitStack

    f32 = mybir.dt.float32
    f32r = mybir.dt.float32r
    bf16 = mybir.dt.bfloat16
    AF = mybir.ActivationFunctionType

    nc = bacc.Bacc()

    x1t = nc.dram_tensor("x1t", [C, N], bf16, kind="ExternalInput")
    x2t = nc.dram_tensor("x2t", [C, N], bf16, kind="ExternalInput")
    xqt = nc.dram_tensor("xqt", [C, NCH], bf16, kind="ExternalInput")
    w2d = nc.dram_tensor("w2", [2, 2, C, C], bf16, kind="ExternalInput")
    qwt = nc.dram_tensor("qwt", [C, C], bf16, kind="ExternalInput")
    kwt = nc.dram_tensor("kwt", [C, C], bf16, kind="ExternalInput")
    vwt = nc.dram_tensor("vwt", [C, C], bf16, kind="ExternalInput")
    pwt = nc.dram_tensor("pwt", [C, C], bf16, kind="ExternalInput")
    brows = nc.dram_tensor("brows", [4, C], bf16, kind="ExternalInput")
    blkd = nc.dram_tensor("blkd", [2, 128, 128], bf16, kind="ExternalInput")
    outt = nc.dram_tensor("outt", [C, NCH], f32, kind="ExternalOutput")

    with nc.allow_low_precision(reason="bf16 matmul inputs; accumulation stays fp32 in PSUM"), tile.TileContext(nc) as tc:
      with tc.tile_pool(name="pg", bufs=1) as PG, \
           tc.tile_pool(name="psum", bufs=1, space="PSUM") as PS:
        # ---- global weights / consts ----
        wsb = {}
        for nm, dram in (("q", qwt), ("k", kwt), ("v", vwt), ("p", pwt)):
            for ch in range(2):
                t = PG.tile([128, C], bf16, name=f"w{nm}{ch}", tag=f"w{nm}{ch}")
                nc.gpsimd.dma_start(out=t, in_=dram[ch * 128:(ch + 1) * 128, :])
                wsb[(nm, ch)] = t
        brow = []
        for bi in range(4):   # 0: sr_b, 1: bvec_k, 2: proj_b, 3: bvec_v
            t = PG.tile([1, C], bf16, name=f"brow{bi}", tag=f"brow{bi}")
            nc.gpsimd.dma_start(out=t, in_=brows[bi:bi + 1, :])
            brow.append(t)
        ones1 = PG.tile([1, 128], bf16, name="ones1", tag="ones1")
        nc.vector.memset(ones1, 1.0)
        ones512 = PG.tile([1, 512], bf16, name="ones512", tag="ones512")
        nc.vector.memset(ones512, 1.0)
        sel2 = PG.tile([128, 4], bf16, name="sel2", tag="sel2")
        nc.vector.memset(sel2, 0.0)
        nc.vector.memset(sel2[:, 0:1], 1.0)
        sel8 = PG.tile([128, 8, 8], bf16, name="sel8", tag="sel8")
        nc.vector.memset(sel8, 0.0)
        for h in range(8):
            nc.vector.memset(sel8[:, h, h:h + 1], 1.0)
        epsT = PG.tile([1, 1], f32, name="epsT", tag="epsT")
        nc.vector.memset(epsT, EPS)
        blk = []
        for grp in range(2):
            t = PG.tile([128, 128], bf16, name=f"blk{grp}", tag=f"blk{grp}")
            nc.gpsimd.dma_start(out=t, in_=blkd[grp])
            blk.append(t)
        k2 = [PG.tile([128, M], bf16, name=f"k2{oh}", tag=f"k2{oh}")
              for oh in range(2)]
        v1 = [PG.tile([128, C], bf16, name=f"v1_{ms}", tag=f"v1_{ms}")
              for ms in range(8)]
        qT = [PG.tile([128, NCH], bf16, name=f"qT{oh}", tag=f"qT{oh}")
              for oh in range(2)]

        # ================= phase 1: conv + LN + kv + q =================
        if True:
            P1 = PG
            xT = {}
            for inp, dram in ((0, x1t), (1, x2t)):
                for ch in range(2):
                    t = P1.tile([128, N], bf16, name=f"x{inp}{ch}", tag=f"x{inp}{ch}")
                    nc.gpsimd.dma_start(out=t, in_=dram[ch * 128:(ch + 1) * 128, :])
                    xT[(inp, ch)] = t
            xq = []
            for ch in range(2):
                t = P1.tile([128, NCH], bf16, name=f"xq{ch}", tag=f"xq{ch}")
                nc.gpsimd.dma_start(out=t, in_=xqt[ch * 128:(ch + 1) * 128, :])
                xq.append(t)
            w2 = []
            for ch in range(2):
                t = P1.tile([128, 2, 2, C], bf16, name=f"w2{ch}", tag=f"w2{ch}")
                nc.gpsimd.dma_start(
                    out=t,
                    in_=w2d[:, :, ch * 128:(ch + 1) * 128, :].rearrange(
                        "kh kw c o -> c kh kw o"))
                w2.append(t)

            z = {}
            for inp in range(2):
                xr = [P1.tile([128, M], bf16, name=f"xr{oh}", tag=f"xr{oh}",
                              bufs=2) for oh in range(2)]
                for oh in range(2):
                    for mh in range(2):
                        ps = PS.tile([128, 512], f32, name="conv", tag="mm",
                                     bufs=1)
                        k = 0
                        for ch in range(2):
                            xv = xT[(inp, ch)].rearrange(
                                "p (i ki j kj) -> p ki kj i j", ki=2, kj=2, j=32)
                            for kh in range(2):
                                for kw in range(2):
                                    nc.tensor.matmul(
                                        ps,
                                        w2[ch][:, kh, kw, oh * 128:(oh + 1) * 128],
                                        xv[:, kh, kw, mh * 16:(mh + 1) * 16, :],
                                        start=(k == 0), stop=False)
                                    k += 1
                        nc.tensor.matmul(
                            ps, brow[0][:, oh * 128:(oh + 1) * 128], ones512,
                            start=False, stop=True)
                        nc.vector.tensor_copy(
                            out=xr[oh][:, mh * 512:(mh + 1) * 512], in_=ps)
                # column stats + LN, per m-half (keeps PSUM tiles 1 bank)
                mean = P1.tile([1, M], bf16, name="mean", tag="mean")
                rstd = P1.tile([1, M], bf16, name="rstd", tag="rstd")
                for ch in range(2):
                    z[(inp, ch)] = P1.tile([128, M], bf16, name=f"z{inp}{ch}",
                                           tag=f"z{inp}{ch}")
                for mh in range(2):
                    sl = slice(mh * 512, (mh + 1) * 512)
                    pmu = PS.tile([1, 512], f32, name="pmu", tag="sc", bufs=2)
                    psq = PS.tile([1, 512], f32, name="psq", tag="sc", bufs=2)
                    for k, oh in enumerate(range(2)):
                        sq = P1.tile([128, 512], bf16, name="sq", tag="sq",
                                     bufs=2)
                        nc.vector.tensor_mul(sq, xr[oh][:, sl], xr[oh][:, sl])
                        nc.tensor.matmul(pmu, sel2[:, 0:1], xr[oh][:, sl],
                                         start=(k == 0), stop=(k == 1))
                        nc.tensor.matmul(psq, sel2[:, 0:1], sq,
                                         start=(k == 0), stop=(k == 1))
                    mn = P1.tile([1, 512], f32, name="mn", tag="mn")
                    nc.scalar.mul(out=mn, in_=pmu, mul=1.0 / C)
                    nc.vector.tensor_copy(out=mean[:, sl], in_=mn)
                    msq = P1.tile([1, 512], f32, name="msq", tag="msq")
                    nc.scalar.activation(out=msq, in_=mn, func=AF.Square)
                    var = P1.tile([1, 512], f32, name="var", tag="var")
                    nc.scalar.mul(out=var, in_=psq, mul=1.0 / C)
                    nc.vector.tensor_sub(var, var, msq)
                    std = P1.tile([1, 512], f32, name="std", tag="std")
                    nc.scalar.activation(out=std, in_=var, func=AF.Sqrt,
                                         bias=epsT)
                    rsf = P1.tile([1, 512], f32, name="rsf", tag="rsf")
                    nc.vector.reciprocal_approx_fast(out=rsf, in_=std)
                    nc.vector.tensor_copy(out=rstd[:, sl], in_=rsf)
                    # broadcast across partitions via rank-1 matmul
                    muB = PS.tile([128, 512], f32, name="muB", tag="sc",
                                  bufs=2)
                    rB = PS.tile([128, 512], f32, name="rB", tag="sc", bufs=2)
                    nc.tensor.matmul(muB, ones1, mean[:, sl],
                                     start=True, stop=True)
                    nc.tensor.matmul(rB, ones1, rstd[:, sl],
                                     start=True, stop=True)
                    for ch in range(2):
                        zt = z[(inp, ch)]
                        nc.vector.tensor_sub(zt[:, sl], xr[ch][:, sl], muB)
                        nc.vector.tensor_mul(zt[:, sl], zt[:, sl], rB)
            # ---- k2T from z(x2) ----
            for oh in range(2):
                for mh in range(2):
                    ps = PS.tile([128, 512], f32, name="kvp", tag="mm", bufs=1)
                    for ch in range(2):
                        nc.tensor.matmul(
                            ps, wsb[("k", ch)][:, oh * 128:(oh + 1) * 128],
                            z[(1, ch)][:, mh * 512:(mh + 1) * 512],
                            start=(ch == 0), stop=False)
                    nc.tensor.matmul(
                        ps, brow[1][:, oh * 128:(oh + 1) * 128], ones512,
                        start=False, stop=True)
                    nc.vector.tensor_copy(
                        out=k2[oh][:, mh * 512:(mh + 1) * 512], in_=ps)
            # ---- v1 natural from z(x1) ----
            for ms in range(8):
                ps = PS.tile([128, C], f32, name="vp", tag="mm", bufs=1)
                for ch in range(2):
                    nc.tensor.matmul(
                        ps, z[(0, ch)][:, ms * 128:(ms + 1) * 128],
                        wsb[("v", ch)], start=(ch == 0), stop=False)
                nc.tensor.matmul(ps, ones1, brow[3],
                                 start=False, stop=True)
                nc.vector.tensor_copy(out=v1[ms], in_=ps)
            # ---- qT ----
            for oh in range(2):
                for nh2 in range(2):
                    ps = PS.tile([128, 512], f32, name="qp", tag="mm", bufs=1)
                    for ch in range(2):
                        nc.tensor.matmul(
                            ps, wsb[("q", ch)][:, oh * 128:(oh + 1) * 128],
                            xq[ch][:, nh2 * 512:(nh2 + 1) * 512],
                            start=(ch == 0), stop=(ch == 1))
                    nc.vector.tensor_copy(
                        out=qT[oh][:, nh2 * 512:(nh2 + 1) * 512], in_=ps)

        # ================= phase 2: attention + proj =================
        if True:
            PA = PG
            for nh2 in range(2):
                nsl = slice(nh2 * 512, (nh2 + 1) * 512)
                U = [PS.tile([128, 512], f32, name=f"U{g}", tag=f"U{g}")
                     for g in range(2)]
                pden = PS.tile([128, 512], f32, name="pden", tag="pden")
                for ms in range(8):
                    for grp in range(2):
                        scs = []
                        for pr in range(2):
                            scps = PS.tile([128, 1024], f32, name="scps",
                                           tag="sc", bufs=2)
                            for i in range(2):
                                h = grp * 4 + pr * 2 + i
                                hb = 32 * (h % 4)
                                nc.tensor.matmul(
                                    scps[:, i * 512:(i + 1) * 512],
                                    k2[h // 4][hb:hb + 32,
                                               ms * 128:(ms + 1) * 128],
                                    qT[h // 4][hb:hb + 32, nsl],
                                    start=True, stop=True,
                                    tile_position=(hb, 0))
                            scs.append(scps)
                        ets = []
                        for pr in range(2):
                            et = PA.tile([128, 1024], bf16, name="et",
                                         tag="et", bufs=4)
                            nc.scalar.activation(out=et, in_=scs[pr],
                                                 func=AF.Exp)
                            ets.append(et)
                        for pr in range(2):
                            for i in range(2):
                                h = grp * 4 + pr * 2 + i
                                h4 = pr * 2 + i
                                esl = ets[pr][:, i * 512:(i + 1) * 512]
                                nc.tensor.matmul(
                                    U[grp][32 * h4:32 * h4 + 32, :],
                                    v1[ms][:, 32 * h:32 * h + 32], esl,
                                    start=(ms == 0), stop=(ms == 7),
                                    tile_position=(0, 32 * h4),
                                    skip_group_check=True)
                        for pr in range(2):
                            for i in range(2):
                                h = grp * 4 + pr * 2 + i
                                g = h % 4
                                esl = ets[pr][:, i * 512:(i + 1) * 512]
                                nc.tensor.matmul(
                                    pden[32 * g:32 * g + 8, :],
                                    sel8[:, h // 4, :], esl,
                                    start=(ms == 0 and grp == 0),
                                    stop=(ms == 7 and grp == 1),
                                    tile_position=(0, 32 * g),
                                    skip_group_check=True)
                pdenS = PA.tile([128, 512], bf16, name="pdenS", tag="pdenS")
                nc.vector.tensor_copy(out=pdenS, in_=pden)
                for grp in range(2):
                    rps = PS.tile([128, 512], f32, name="rps", tag="pden")
                    nc.tensor.matmul(rps, blk[grp], pdenS,
                                     start=True, stop=True)
                    recf = PA.tile([128, 512], f32, name=f"recf{grp}",
                                   tag=f"recf{grp}")
                    nc.vector.reciprocal_approx_fast(out=recf, in_=rps)
                    ot = PA.tile([128, 512], bf16, name=f"oT{grp}",
                                 tag=f"oT{grp}")
                    nc.vector.tensor_mul(ot, U[grp], recf)
                    oT = ot if grp else [ot]
                    if grp == 0:
                        oT = [ot]
                    else:
                        oT = [oT0, ot]
                    oT0 = ot if grp == 0 else oT0
                for oh in range(2):
                    ps = PS.tile([128, 512], f32, name="pj", tag="mm", bufs=1)
                    for ch in range(2):
                        nc.tensor.matmul(
                            ps, wsb[("p", ch)][:, oh * 128:(oh + 1) * 128],
                            oT[ch], start=(ch == 0), stop=False)
                    nc.tensor.matmul(
                        ps, brow[2][:, oh * 128:(oh + 1) * 128], ones512,
                        start=False, stop=True)
                    y = PA.tile([128, 512], f32, name="y", tag="y", bufs=2)
                    nc.vector.tensor_copy(out=y, in_=ps)
                    nc.gpsimd.dma_start(out=outt[oh * 128:(oh + 1) * 128, nsl],
                                      in_=y)
    nc.finalize()
    return nc


def _get_program():
    if "nc" not in _prog_cache:
        _prog_cache["nc"] = _build_program()
    return _prog_cache["nc"]


def kernel(x1, x2, q_w, kv_w, sr_w, sr_b, ln_g, ln_b, proj_w, proj_b,
           H1=64, W1=64, H2=64, W2=64, **_):
    from concourse.bass_utils import run_bass_kernel_spmd

    f = np.float32
    x1 = np.asarray(x1, f)
    x2 = np.asarray(x2, f)
    q_w = np.asarray(q_w, f)
    kv_w = np.asarray(kv_w, f)
    sr_w = np.asarray(sr_w, f)
    sr_b = np.asarray(sr_b, f)
    ln_g = np.asarray(ln_g, f)
    ln_b = np.asarray(ln_b, f)
    proj_w = np.asarray(proj_w, f)
    proj_b = np.asarray(proj_b, f)

    import ml_dtypes
    bf = ml_dtypes.bfloat16
    qwT = np.ascontiguousarray(q_w.T * SCALE)
    kwT = np.ascontiguousarray(ln_g[:, None] * kv_w[:C].T)
    vwT = np.ascontiguousarray(ln_g[:, None] * kv_w[C:].T)
    bvec_k = kv_w[:C] @ ln_b
    bvec_v = kv_w[C:] @ ln_b
    pwT = np.ascontiguousarray(proj_w.T)
    w2 = np.ascontiguousarray(sr_w.transpose(2, 3, 1, 0))
    brows = np.ascontiguousarray(
        np.stack([sr_b, bvec_k, proj_b, bvec_v], axis=0))
    x1T = [np.ascontiguousarray(x1[b].T).astype(bf) for b in range(B)]
    x2T = [np.ascontiguousarray(x2[b].T).astype(bf) for b in range(B)]
    qwT = qwT.astype(bf)
    kwT = kwT.astype(bf)
    vwT = vwT.astype(bf)
    pwT = pwT.astype(bf)
    w2 = w2.astype(bf)
    blkd = np.zeros((2, 128, 128), bf)
    for grp in range(2):
        for i in range(128):
            h = grp * 4 + i // 32
            src_row = 32 * (h % 4) + h // 4
            blkd[grp, src_row, i] = 1.0

    in_maps = []
    for core in range(8):
        b, chk = divmod(core, 4)
        in_maps.append({
            "x1t": x1T[b], "x2t": x2T[b],
            "xqt": np.ascontiguousarray(x1T[b][:, chk * NCH:(chk + 1) * NCH]),
            "w2": w2, "qwt": qwT, "kwt": kwT, "vwt": vwT, "pwt": pwT,
            "brows": brows.astype(bf), "blkd": blkd,
        })

    nc = _get_program()
    res = run_bass_kernel_spmd(nc, in_maps, core_ids=list(range(8)))
    out = np.empty((B, N, C), f)
    for core in range(8):
        b, chk = divmod(core, 4)
        out[b, chk * NCH:(chk + 1) * NCH, :] = res.results[core]["outt"].T
    return out



# revision 11
# speedup vs baseline: 1.0951x; 1.0951x over previous
"""CrossTemporalAttention2 Trainium2 kernel (pipelined rewrite, v2).

Sharding: 8 cores = 2 batches x 4 query-chunks of 1024 rows.
Each core: conv+LN+KV for its batch (duplicated across the 4 cores of the
batch group), attention + proj for its 1024 query rows.

Key structure:
  - conv/LN-stats/k2/v1 are emitted in m-chunks interleaved into the
    attention loop, so the ACT engine (exp is the hard floor at ~55us)
    starts early and Tensor fills its gaps with conv work.
  - LayerNorm is never materialized: k2u = kwT@(xr-mu) via a rank-1
    correction matmul, and the per-m rstd is folded into the softmax exp's
    per-partition `scale`; on the v side rstd is applied as a per-partition
    tensor_scalar multiply during the PSUM->SBUF copy. rstd itself comes
    from a DVE Newton rsqrt on variance columns (no sqrt/ln on ACT ->
    single activation table set, zero table switches).
  - k-side LN beta adds a per-n constant to scores, which softmax cancels
    exactly -> dropped. v-side LN beta is folded into the proj bias on the
    host. conv bias is added during the conv PSUM->SBUF copy (per-partition
    tensor_scalar add).
  - scores: 2 heads per PSUM slot [128,1024], row-tiled (tile_position=
    (32j,0)) so the pair runs concurrently in the PE array; one exp per
    pair; U (AV) + denominator matmuls form a 4-up col-tiled concurrent
    set (U at cols 32j, den at complementary cols).
  - denominator rows accumulate in one PSUM bank (full 32-row windows so
    no PSUM garbage is ever read); broadcast per head via a host-built
    permutation matmul; reciprocal on DVE.
  - n2=0 defers U/den emission by a few steps (software lag) so v1[ms]
    production is always emitted before its consumers.

PSUM (8 banks): U0,U1,pden (3) + scores 2x[128,1024] (4) + util (1).
"""

import numpy as np

B, N, C = 2, 4096, 256
H, Dh = 8, 32
M = 1024          # (64/2) * (64/2)
NCH = 1024        # query rows per core
SCALE = Dh ** -0.5
EPS = 1e-5

_prog_cache = {}


def _build_program():
    import concourse.bass as bass
    import concourse.bacc as bacc
    import concourse.tile as tile
    from concourse import mybir

    f32 = mybir.dt.float32
    bf16 = mybir.dt.bfloat16
    AF = mybir.ActivationFunctionType
    OP = mybir.AluOpType

    nc = bacc.Bacc()

    x1t = nc.dram_tensor("x1t", [C, N], bf16, kind="ExternalInput")
    x2t = nc.dram_tensor("x2t", [C, N], bf16, kind="ExternalInput")
    xqt = nc.dram_tensor("xqt", [C, NCH], bf16, kind="ExternalInput")
    w2d = nc.dram_tensor("w2", [2, 2, C, C], bf16, kind="ExternalInput")
    qwt = nc.dram_tensor("qwt", [C, C], bf16, kind="ExternalInput")
    kwt = nc.dram_tensor("kwt", [C, C], bf16, kind="ExternalInput")
    vwt = nc.dram_tensor("vwt", [C, C], bf16, kind="ExternalInput")
    pwt = nc.dram_tensor("pwt", [C, C], bf16, kind="ExternalInput")
    pbrow = nc.dram_tensor("pbrow", [1, C], bf16, kind="ExternalInput")
    bksd = nc.dram_tensor("bksd", [1, C], bf16, kind="ExternalInput")
    bvsd = nc.dram_tensor("bvsd", [1, C], bf16, kind="ExternalInput")
    srb2 = nc.dram_tensor("srb2", [128, 2], f32, kind="ExternalInput")
    blkd = nc.dram_tensor("blkd", [2, 128, 128], bf16, kind="ExternalInput")
    outt = nc.dram_tensor("outt", [C, NCH], f32, kind="ExternalOutput")

    with nc.allow_low_precision(reason="bf16 matmul inputs; fp32 PSUM accum"), \
         tile.TileContext(nc) as tc:
      with tc.tile_pool(name="pg", bufs=1) as PG, \
           tc.tile_pool(name="psum", bufs=1, space="PSUM") as PS:

        # ----- global weights / consts (DMA'd in priority order) -----
        w2s = [PG.tile([128, 2, 2, C], bf16, name=f"w2{ch}", tag=f"w2{ch}")
               for ch in range(2)]
        wq = [PG.tile([128, C], bf16, name=f"wq{ch}", tag=f"wq{ch}")
              for ch in range(2)]
        wk = [PG.tile([128, C], bf16, name=f"wk{ch}", tag=f"wk{ch}")
              for ch in range(2)]
        wv = [PG.tile([128, C], bf16, name=f"wv{ch}", tag=f"wv{ch}")
              for ch in range(2)]
        wp = [PG.tile([128, C], bf16, name=f"wp{ch}", tag=f"wp{ch}")
              for ch in range(2)]
        x2s = [PG.tile([128, N], bf16, name=f"x2s{ch}", tag=f"x2s{ch}")
               for ch in range(2)]
        x1s = [PG.tile([128, N], bf16, name=f"x1s{ch}", tag=f"x1s{ch}")
               for ch in range(2)]
        xq = [PG.tile([128, NCH], bf16, name=f"xq{ch}", tag=f"xq{ch}")
              for ch in range(2)]
        pbr = PG.tile([1, C], bf16, name="pbr", tag="pbr")
        bks = PG.tile([1, C], bf16, name="bks", tag="bks")
        bvs = PG.tile([1, C], bf16, name="bvs", tag="bvs")
        srb = PG.tile([128, 2], f32, name="srb", tag="srb")
        blk = [PG.tile([128, 128], bf16, name=f"blk{g}", tag=f"blk{g}")
               for g in range(2)]

        nc.gpsimd.dma_start(out=w2s[0], in_=w2d[:, :, 0:128, :].rearrange(
            "kh kw c o -> c kh kw o"))
        nc.gpsimd.dma_start(out=w2s[1], in_=w2d[:, :, 128:256, :].rearrange(
            "kh kw c o -> c kh kw o"))
        for ch in range(2):
            nc.gpsimd.dma_start(out=x2s[ch], in_=x2t[ch * 128:(ch + 1) * 128, 0:N])
        for ch in range(2):
            nc.gpsimd.dma_start(out=wk[ch], in_=kwt[ch * 128:(ch + 1) * 128, :])
        for ch in range(2):
            nc.gpsimd.dma_start(out=wq[ch], in_=qwt[ch * 128:(ch + 1) * 128, :])
        for ch in range(2):
            nc.gpsimd.dma_start(out=xq[ch], in_=xqt[ch * 128:(ch + 1) * 128, :])
        nc.gpsimd.dma_start(out=srb, in_=srb2[:, :])
        nc.gpsimd.dma_start(out=bks, in_=bksd[:, :])
        for ch in range(2):
            nc.gpsimd.dma_start(out=x1s[ch], in_=x1t[ch * 128:(ch + 1) * 128, 0:N])
        for ch in range(2):
            nc.gpsimd.dma_start(out=wv[ch], in_=vwt[ch * 128:(ch + 1) * 128, :])
        nc.gpsimd.dma_start(out=bvs, in_=bvsd[:, :])
        for ch in range(2):
            nc.gpsimd.dma_start(out=wp[ch], in_=pwt[ch * 128:(ch + 1) * 128, :])
        nc.gpsimd.dma_start(out=pbr, in_=pbrow[:, :])
        for g in range(2):
            nc.gpsimd.dma_start(out=blk[g], in_=blkd[g])

        onescol = PG.tile([128, 1], bf16, name="onescol", tag="onescol")
        nc.vector.memset(onescol, 1.0)
        one11 = PG.tile([1, 1], f32, name="one11", tag="one11")
        nc.vector.memset(one11, 1.0)
        ones512 = PG.tile([1, 512], bf16, name="ones512", tag="ones512")
        nc.vector.memset(ones512, 1.0)
        selden = PG.tile([128, 2, 32], bf16, name="selden", tag="selden")
        nc.vector.memset(selden, 0.0)
        for g in range(2):
            nc.vector.memset(selden[:, g, g:g + 1], 1.0)

        # ----- persistent SBUF products -----
        k2 = [PG.tile([128, M], bf16, name=f"k2{g}", tag=f"k2{g}")
              for g in range(2)]
        v1 = [PG.tile([128, C], bf16, name=f"v1_{ms}", tag=f"v1_{ms}")
              for ms in range(8)]
        qTs = [[PG.tile([128, 512], bf16, name=f"qT{n}{g}", tag=f"qT{n}{g}")
                for g in range(2)] for n in range(2)]
        rstdc = [PG.tile([128, 8], f32, name=f"rstdc{i}", tag=f"rstdc{i}")
                 for i in range(2)]   # [0]=x1 (v side), [1]=x2 (k side)

        # ---------- phase-1 helpers ----------
        def conv_half(xs, base, w, oh):
            """conv for m-cols [base, base+w), out-ch chunk oh -> biased
            bf16 SBUF tile."""
            i0, ni = base // 32, w // 32
            ps = PS.tile([128, 512], f32, name="cnv", tag="util", bufs=1)
            k = 0
            for ch in range(2):
                xv = xs[ch].rearrange("p (i ki j kj) -> p ki kj i j",
                                      ki=2, kj=2, j=32)
                for kh in range(2):
                    for kw in range(2):
                        nc.tensor.matmul(
                            ps[:, 0:w],
                            w2s[ch][:, kh, kw, oh * 128:(oh + 1) * 128],
                            xv[:, kh, kw, i0:i0 + ni, :],
                            start=(k == 0), stop=(k == 7))
                        k += 1
            xr = PG.tile([128, 512], bf16, name="xr", tag=f"xr{oh}", bufs=3)
            nc.vector.tensor_scalar_add(out=xr[:, 0:w], in0=ps[:, 0:w],
                                        scalar1=srb[:, oh:oh + 1])
            return xr

        def stats_rows(xrs, w):
            """row stats for a conv chunk: returns (mnneg row bf16, var row
            f32)."""
            sq = [PG.tile([128, 512], bf16, name="sqt", tag=f"sq{ch}", bufs=2)
                  for ch in range(2)]
            for ch in range(2):
                nc.vector.tensor_mul(sq[ch][:, 0:w], xrs[ch][:, 0:w],
                                     xrs[ch][:, 0:w])
            st = PS.tile([128, 512], f32, name="st", tag="util", bufs=1)
            for ch in range(2):
                nc.tensor.matmul(st[0:1, 0:w], onescol, xrs[ch][:, 0:w],
                                 start=(ch == 0), stop=(ch == 1),
                                 tile_position=(0, 0), skip_group_check=True)
                nc.tensor.matmul(st[32:33, 0:w], onescol, sq[ch][:, 0:w],
                                 start=(ch == 0), stop=(ch == 1),
                                 tile_position=(0, 32), skip_group_check=True)
            mnneg = PG.tile([1, 512], bf16, name="mnneg", tag="mnneg", bufs=3)
            nc.vector.tensor_scalar_mul(out=mnneg[:, 0:w], in0=st[0:1, 0:w],
                                        scalar1=-1.0 / C)
            psqs = PG.tile([1, 512], f32, name="psqs", tag="psqs", bufs=2)
            nc.vector.tensor_scalar(out=psqs[:, 0:w], in0=st[32:33, 0:w],
                                    scalar1=1.0 / C, scalar2=EPS,
                                    op0=OP.mult, op1=OP.add)
            msq = PG.tile([1, 512], f32, name="msq", tag="msq", bufs=2)
            nc.vector.tensor_mul(msq[:, 0:w], mnneg[:, 0:w], mnneg[:, 0:w])
            var = PG.tile([1, 512], f32, name="var", tag="var", bufs=2)
            nc.vector.tensor_sub(var[:, 0:w], psqs[:, 0:w], msq[:, 0:w])
            return mnneg, var

        def stats_cols(var, inp, base, w):
            """transpose var row -> psum columns, Newton rsqrt -> rstdc."""
            nq = w // 128
            vc = PS.tile([128, 512], f32, name="vc", tag="util", bufs=1)
            for q in range(nq):
                nc.tensor.matmul(vc[:, q:q + 1],
                                 var[:, q * 128:(q + 1) * 128], one11,
                                 start=True, stop=True,
                                 skip_group_check=True)
            r = PG.tile([128, 8], f32, name="rr", tag="rr", bufs=2)
            nc.vector.reciprocal_approx_fast(out=r[:, 0:nq], in_=vc[:, 0:nq])
            x = rstdc[inp][:, base // 128:base // 128 + nq]
            nc.vector.tensor_scalar(out=x, in0=r[:, 0:nq],
                                    scalar1=0.537, scalar2=0.340,
                                    op0=OP.mult, op1=OP.add)
            s = PG.tile([128, 8], f32, name="ss", tag="ss", bufs=2)
            t = PG.tile([128, 8], f32, name="tt", tag="tt", bufs=2)
            u = PG.tile([128, 8], f32, name="uu", tag="uu", bufs=2)
            for _ in range(2):
                nc.vector.tensor_mul(s[:, 0:nq], x, x)
                nc.vector.tensor_mul(t[:, 0:nq], s[:, 0:nq], vc[:, 0:nq])
                nc.vector.tensor_scalar(out=u[:, 0:nq], in0=t[:, 0:nq],
                                        scalar1=-0.5, scalar2=1.5,
                                        op0=OP.mult, op1=OP.add)
                nc.vector.tensor_mul(x, x, u[:, 0:nq])

        def k2_chunk(xrs, mnneg, base, w):
            for g in range(2):
                ps = PS.tile([128, 512], f32, name="k2p", tag="util", bufs=1)
                for ch in range(2):
                    nc.tensor.matmul(
                        ps[:, 0:w], wk[ch][:, g * 128:(g + 1) * 128],
                        xrs[ch][:, 0:w], start=(ch == 0), stop=False)
                nc.tensor.matmul(ps[:, 0:w], bks[:, g * 128:(g + 1) * 128],
                                 mnneg[:, 0:w], start=False, stop=True)
                nc.vector.tensor_copy(out=k2[g][:, base:base + w],
                                      in_=ps[:, 0:w])

        def v1_chunk(ms, xrs, mnneg, cbase):
            """v1[ms] from x1 conv chunk starting at cbase (tiles xrs)."""
            off = ms * 128 - cbase
            ps = PS.tile([128, 512], f32, name="v1p", tag="util", bufs=1)
            for ch in range(2):
                nc.tensor.matmul(
                    ps[:, 0:C], xrs[ch][:, off:off + 128], wv[ch],
                    start=(ch == 0), stop=False)
            nc.tensor.matmul(ps[:, 0:C], mnneg[:, off:off + 128], bvs,
                             start=False, stop=True)
            nc.vector.tensor_scalar_mul(out=v1[ms], in0=ps[:, 0:C],
                                        scalar1=rstdc[0][:, ms:ms + 1])

        def qt_chunk(n):
            for g in range(2):
                ps = PS.tile([128, 512], f32, name="qtp", tag="util", bufs=1)
                for ch in range(2):
                    nc.tensor.matmul(
                        ps, wq[ch][:, g * 128:(g + 1) * 128],
                        xq[ch][:, n * 512:(n + 1) * 512],
                        start=(ch == 0), stop=(ch == 1))
                nc.vector.tensor_copy(out=qTs[n][g], in_=ps)

        def conv_x2(mq):
            xrs = [conv_half(x2s, mq * 256, 256, oh) for oh in range(2)]
            mnneg, var = stats_rows(xrs, 256)
            k2_chunk(xrs, mnneg, mq * 256, 256)
            stats_cols(var, 1, mq * 256, 256)

        x1state = {}

        def conv_x1(mh):
            xrs = [conv_half(x1s, mh * 512, 512, oh) for oh in range(2)]
            mnneg, var = stats_rows(xrs, 512)
            stats_cols(var, 0, mh * 512, 512)
            x1state[mh] = (xrs, mnneg)

        def v1_pair(a):
            mh = a // 2
            xrs, mnneg = x1state[mh]
            v1_chunk(2 * a, xrs, mnneg, mh * 512)
            v1_chunk(2 * a + 1, xrs, mnneg, mh * 512)

        work = [
            lambda: conv_x1(0),
            lambda: v1_pair(0),
            lambda: conv_x2(1),
            lambda: v1_pair(1),
            lambda: conv_x1(1),
            lambda: v1_pair(2),
            lambda: conv_x2(2),
            lambda: v1_pair(3),
            lambda: conv_x2(3),
            lambda: qt_chunk(1),
        ]
        widx = [0]

        def drain_one():
            if widx[0] < len(work):
                work[widx[0]]()
                widx[0] += 1

        # ---------------- prologue ----------------
        qt_chunk(0)
        conv_x2(0)

        # ---------------- attention ----------------
        for n2 in range(2):
            U = [PS.tile([128, 512], f32, name=f"U{g}", tag=f"U{g}", bufs=1)
                 for g in range(2)]
            pden = PS.tile([128, 512], f32, name="pden", tag="pden", bufs=1)

            def emit_ud(E, ms, grp, pr):
                for i in range(2):
                    j = 2 * pr + i
                    h = grp * 4 + j
                    nc.tensor.matmul(
                        U[grp][32 * j:32 * j + 32, :],
                        v1[ms][:, 32 * h:32 * h + 32],
                        E[:, i * 512:(i + 1) * 512],
                        start=(ms == 0), stop=(ms == 7),
                        tile_position=(0, 32 * j),
                        skip_group_check=True)
                for i in range(2):
                    j = 2 * pr + i
                    cpos = 32 * ((j + 2) % 4)
                    nc.tensor.matmul(
                        pden[cpos:cpos + 32, :],
                        selden[:, grp, :],
                        E[:, i * 512:(i + 1) * 512],
                        start=(ms == 0 and grp == 0),
                        stop=(ms == 7 and grp == 1),
                        tile_position=(0, cpos),
                        skip_group_check=True)

            steps = [(ms, grp, pr) for ms in range(8) for grp in range(2)
                     for pr in range(2)]
            lag = 5 if n2 == 0 else 0
            pending = []
            for tstep, (ms, grp, pr) in enumerate(steps):
                S = PS.tile([128, 1024], f32, name="S", tag="scps", bufs=2)
                for i in range(2):
                    j = 2 * pr + i
                    nc.tensor.matmul(
                        S[:, i * 512:(i + 1) * 512],
                        k2[grp][32 * j:32 * j + 32,
                                ms * 128:(ms + 1) * 128],
                        qTs[n2][grp][32 * j:32 * j + 32, :],
                        start=True, stop=True,
                        tile_position=(32 * j, 0))
                E = PG.tile([128, 1024], bf16, name="E", tag="E", bufs=7)
                nc.scalar.activation(out=E, in_=S, func=AF.Exp,
                                     scale=rstdc[1][:, ms:ms + 1])
                pending.append((E, ms, grp, pr))
                if n2 == 0 and tstep % 2 == 1:
                    drain_one()
                if len(pending) > lag:
                    emit_ud(*pending.pop(0))
            while pending:
                emit_ud(*pending.pop(0))

            # ---- normalize + proj for this n-chunk ----
            pdenS = PG.tile([128, 512], bf16, name="pdenS", tag="pdenS",
                            bufs=2)
            nc.vector.tensor_copy(out=pdenS, in_=pden)
            ot = []
            for g in range(2):
                rps = PS.tile([128, 512], f32, name="rps", tag="util", bufs=1)
                nc.tensor.matmul(rps, blk[g], pdenS, start=True, stop=True)
                recf = PG.tile([128, 512], f32, name="recf", tag="recf",
                               bufs=2)
                nc.vector.reciprocal_approx_fast(out=recf, in_=rps)
                o = PG.tile([128, 512], bf16, name="ot", tag=f"ot{g}", bufs=2)
                nc.vector.tensor_mul(o, U[g], recf)
                ot.append(o)
            P = PS.tile([128, 1024], f32, name="P", tag="scps", bufs=2)
            for oh in range(2):
                psl = P[:, oh * 512:(oh + 1) * 512]
                for ch in range(2):
                    nc.tensor.matmul(
                        psl, wp[ch][:, oh * 128:(oh + 1) * 128], ot[ch],
                        start=(ch == 0), stop=False)
                nc.tensor.matmul(psl, pbr[:, oh * 128:(oh + 1) * 128],
                                 ones512, start=False, stop=True)
                y = PG.tile([128, 512], f32, name="y", tag="y", bufs=2)
                nc.vector.tensor_copy(out=y, in_=psl)
                nc.gpsimd.dma_start(
                    out=outt[oh * 128:(oh + 1) * 128,
                             n2 * 512:(n2 + 1) * 512],
                    in_=y)
    nc.finalize()
    return nc


def _get_program():
    if "nc" not in _prog_cache:
        _prog_cache["nc"] = _build_program()
    return _prog_cache["nc"]


def kernel(x1, x2, q_w, kv_w, sr_w, sr_b, ln_g, ln_b, proj_w, proj_b,
           H1=64, W1=64, H2=64, W2=64, **_):
    from concourse.bass_utils import run_bass_kernel_spmd

    f = np.float32
    x1 = np.asarray(x1, f)
    x2 = np.asarray(x2, f)
    q_w = np.asarray(q_w, f)
    kv_w = np.asarray(kv_w, f)
    sr_w = np.asarray(sr_w, f)
    sr_b = np.asarray(sr_b, f)
    ln_g = np.asarray(ln_g, f)
    ln_b = np.asarray(ln_b, f)
    proj_w = np.asarray(proj_w, f)
    proj_b = np.asarray(proj_b, f)

    import ml_dtypes
    bf = ml_dtypes.bfloat16
    qwT = np.ascontiguousarray(q_w.T * SCALE).astype(bf)
    kwT = np.ascontiguousarray(ln_g[:, None] * kv_w[:C].T).astype(bf)
    vwT = np.ascontiguousarray(ln_g[:, None] * kv_w[C:].T).astype(bf)
    bksum = (kv_w[:C] @ ln_g)          # column sums of kwT
    bvsum = (kv_w[C:] @ ln_g)          # column sums of vwT
    bvec_v = kv_w[C:] @ ln_b
    pwT = np.ascontiguousarray(proj_w.T).astype(bf)
    w2 = np.ascontiguousarray(sr_w.transpose(2, 3, 1, 0)).astype(bf)
    pbrow = np.ascontiguousarray(
        (proj_b + proj_w @ bvec_v)[None, :]).astype(bf)
    srb2 = np.ascontiguousarray(sr_b.reshape(2, 128).T).astype(f)
    # den row for head (grp, j) sits at pden partition 32*((j+2)%4) + grp;
    # blkd[g] broadcasts it across that head's 32 output rows.
    blkdm = np.zeros((2, 128, 128), bf)
    for g in range(2):
        for i in range(128):
            j = i // 32
            src = 32 * ((j + 2) % 4) + g
            blkdm[g, src, i] = 1.0
    x1T = [np.ascontiguousarray(x1[b].T).astype(bf) for b in range(B)]
    x2T = [np.ascontiguousarray(x2[b].T).astype(bf) for b in range(B)]

    in_maps = []
    for core in range(8):
        b, chk = divmod(core, 4)
        in_maps.append({
            "x1t": x1T[b], "x2t": x2T[b],
            "xqt": np.ascontiguousarray(x1T[b][:, chk * NCH:(chk + 1) * NCH]),
            "w2": w2, "qwt": qwT, "kwt": kwT, "vwt": vwT, "pwt": pwT,
            "pbrow": pbrow, "srb2": srb2, "blkd": blkdm,
            "bksd": np.ascontiguousarray(bksum[None, :]).astype(bf),
            "bvsd": np.ascontiguousarray(bvsum[None, :]).astype(bf),
        })

    nc = _get_program()
    res = run_bass_kernel_spmd(nc, in_maps, core_ids=list(range(8)))
    out = np.empty((B, N, C), f)
    for core in range(8):
        b, chk = divmod(core, 4)
        out[b, chk * NCH:(chk + 1) * NCH, :] = res.results[core]["outt"].T
    return out


# revision 12
# speedup vs baseline: 1.3059x; 1.1925x over previous
"""CrossTemporalAttention2 Trainium2 kernel (pipelined rewrite, v3).

Sharding: 8 cores = 2 batches x 4 query-chunks of 1024 rows.
Each core: conv+LN+KV for its batch (duplicated across the 4 cores of the
batch group), attention + proj for its 1024 query rows.

v3 structure:
  - phase-1 (conv/stats/k2/v1/qT) is flattened into micro work items
    (<=4 matmuls each) drained two per attention block, so the Tensor
    engine stream stays dense (p-state!) and exp starts early.
  - LN is never materialized. k side: k2 = (kwT@xr + (-mu)@bksum) * rstd2
    where rstd2 is a row computed by a DVE Newton rsqrt and applied via a
    rank-1 broadcast matmul + tensor_tensor multiply at the PSUM->SBUF
    copy. v side: rstd1 columns (PE row->col transpose + DVE Newton)
    applied as per-partition tensor_scalar multiply. exp stays scale-free
    (a per-partition scale AP costs ~+50% on the ACT engine).
  - LN beta: k side cancels in softmax exactly; v side folds into proj
    bias on host. conv bias via per-partition tensor_scalar add.
  - attention per (ms, grp) block: 4 score matmuls row-tiled at
    (32j, 0) -> all 4 concurrent in the PE array; 2 exps [128,1024];
    U (AV) + den matmuls form 4-up col-tiled concurrent sets.
  - denominator windows are written full (32 rows) so no uninitialized
    PSUM is ever read; broadcast per head via host permutation matmul.
  - n2=0 defers U/den by `lag` exp-pairs so v1[ms] emission always
    precedes its consumers (E pool holds lag+2 tiles).

PSUM (8 banks): U0,U1,pden (3) + scores 2x[128,1024] (4) + util (1).
"""

import numpy as np

B, N, C = 2, 4096, 256
H, Dh = 8, 32
M = 1024
NCH = 1024
SCALE = Dh ** -0.5
EPS = 1e-5

X2CH = [(0, 256), (256, 512), (768, 256)]   # k-side conv chunks
X1CH = [(0, 512), (512, 512)]               # v-side conv chunks

_prog_cache = {}


def _build_program():
    import concourse.bass as bass
    import concourse.bacc as bacc
    import concourse.tile as tile
    from concourse import mybir

    f32 = mybir.dt.float32
    bf16 = mybir.dt.bfloat16
    AF = mybir.ActivationFunctionType
    OP = mybir.AluOpType

    nc = bacc.Bacc()

    x1t = nc.dram_tensor("x1t", [C, N], bf16, kind="ExternalInput")
    x2t = nc.dram_tensor("x2t", [C, N], bf16, kind="ExternalInput")
    xqt = nc.dram_tensor("xqt", [C, NCH], bf16, kind="ExternalInput")
    w2d = nc.dram_tensor("w2", [2, 2, C, C], bf16, kind="ExternalInput")
    qwt = nc.dram_tensor("qwt", [C, C], bf16, kind="ExternalInput")
    kwt = nc.dram_tensor("kwt", [C, C], bf16, kind="ExternalInput")
    vwt = nc.dram_tensor("vwt", [C, C], bf16, kind="ExternalInput")
    pwt = nc.dram_tensor("pwt", [C, C], bf16, kind="ExternalInput")
    pbrow = nc.dram_tensor("pbrow", [1, C], bf16, kind="ExternalInput")
    bksd = nc.dram_tensor("bksd", [1, C], bf16, kind="ExternalInput")
    bvsd = nc.dram_tensor("bvsd", [1, C], bf16, kind="ExternalInput")
    srb2 = nc.dram_tensor("srb2", [128, 2], f32, kind="ExternalInput")
    blkd = nc.dram_tensor("blkd", [2, 128, 128], bf16, kind="ExternalInput")
    outt = nc.dram_tensor("outt", [C, NCH], f32, kind="ExternalOutput")

    with nc.allow_low_precision(reason="bf16 matmul inputs; fp32 PSUM accum"), \
         tile.TileContext(nc) as tc:
      with tc.tile_pool(name="pg", bufs=1) as PG, \
           tc.tile_pool(name="psum", bufs=1, space="PSUM") as PS:

        w2s = [PG.tile([128, 2, 2, C], bf16, name=f"w2{ch}", tag=f"w2{ch}")
               for ch in range(2)]
        wq = [PG.tile([128, C], bf16, name=f"wq{ch}", tag=f"wq{ch}")
              for ch in range(2)]
        wk = [PG.tile([128, C], bf16, name=f"wk{ch}", tag=f"wk{ch}")
              for ch in range(2)]
        wv = [PG.tile([128, C], bf16, name=f"wv{ch}", tag=f"wv{ch}")
              for ch in range(2)]
        wp = [PG.tile([128, C], bf16, name=f"wp{ch}", tag=f"wp{ch}")
              for ch in range(2)]
        x2s = [PG.tile([128, N], bf16, name=f"x2s{ch}", tag=f"x2s{ch}")
               for ch in range(2)]
        x1s = [PG.tile([128, N], bf16, name=f"x1s{ch}", tag=f"x1s{ch}")
               for ch in range(2)]
        xq = [PG.tile([128, NCH], bf16, name=f"xq{ch}", tag=f"xq{ch}")
              for ch in range(2)]
        pbr = PG.tile([1, C], bf16, name="pbr", tag="pbr")
        bks = PG.tile([1, C], bf16, name="bks", tag="bks")
        bvs = PG.tile([1, C], bf16, name="bvs", tag="bvs")
        srb = PG.tile([128, 2], f32, name="srb", tag="srb")
        blk = [PG.tile([128, 128], bf16, name=f"blk{g}", tag=f"blk{g}")
               for g in range(2)]

        # DMA priority order: conv-c0 critical path first.
        nc.gpsimd.dma_start(out=w2s[0], in_=w2d[:, :, 0:128, :].rearrange(
            "kh kw c o -> c kh kw o"))
        nc.gpsimd.dma_start(out=w2s[1], in_=w2d[:, :, 128:256, :].rearrange(
            "kh kw c o -> c kh kw o"))
        for ch in range(2):   # conv input for x2 chunk c0 ([0:256) -> cols 0:1024)
            nc.gpsimd.dma_start(out=x2s[ch][:, 0:1024],
                                in_=x2t[ch * 128:(ch + 1) * 128, 0:1024])
        nc.gpsimd.dma_start(out=srb, in_=srb2[:, :])
        nc.gpsimd.dma_start(out=bks, in_=bksd[:, :])
        for ch in range(2):
            nc.gpsimd.dma_start(out=wk[ch], in_=kwt[ch * 128:(ch + 1) * 128, :])
        for ch in range(2):
            nc.gpsimd.dma_start(out=wq[ch], in_=qwt[ch * 128:(ch + 1) * 128, :])
        for ch in range(2):
            nc.gpsimd.dma_start(out=xq[ch], in_=xqt[ch * 128:(ch + 1) * 128, :])
        for ch in range(2):   # x2 chunk c1 ([256:768) -> cols 1024:3072)
            nc.gpsimd.dma_start(out=x2s[ch][:, 1024:3072],
                                in_=x2t[ch * 128:(ch + 1) * 128, 1024:3072])
        for ch in range(2):   # x1 chunk mh0 (cols 0:2048)
            nc.gpsimd.dma_start(out=x1s[ch][:, 0:2048],
                                in_=x1t[ch * 128:(ch + 1) * 128, 0:2048])
        for ch in range(2):
            nc.gpsimd.dma_start(out=wv[ch], in_=vwt[ch * 128:(ch + 1) * 128, :])
        nc.gpsimd.dma_start(out=bvs, in_=bvsd[:, :])
        for ch in range(2):   # x2 chunk c2 (cols 3072:4096)
            nc.gpsimd.dma_start(out=x2s[ch][:, 3072:4096],
                                in_=x2t[ch * 128:(ch + 1) * 128, 3072:4096])
        for ch in range(2):   # x1 chunk mh1 (cols 2048:4096)
            nc.gpsimd.dma_start(out=x1s[ch][:, 2048:4096],
                                in_=x1t[ch * 128:(ch + 1) * 128, 2048:4096])
        for ch in range(2):
            nc.gpsimd.dma_start(out=wp[ch], in_=pwt[ch * 128:(ch + 1) * 128, :])
        nc.gpsimd.dma_start(out=pbr, in_=pbrow[:, :])
        for g in range(2):
            nc.gpsimd.dma_start(out=blk[g], in_=blkd[g])

        onescol = PG.tile([128, 1], bf16, name="onescol", tag="onescol")
        nc.vector.memset(onescol, 1.0)
        one11 = PG.tile([1, 1], f32, name="one11", tag="one11")
        nc.vector.memset(one11, 1.0)
        ones1 = PG.tile([1, 128], bf16, name="ones1", tag="ones1")
        nc.vector.memset(ones1, 1.0)
        ones512 = PG.tile([1, 512], bf16, name="ones512", tag="ones512")
        nc.vector.memset(ones512, 1.0)
        selden = PG.tile([128, 2, 32], bf16, name="selden", tag="selden")
        nc.vector.memset(selden, 0.0)
        for g in range(2):
            nc.vector.memset(selden[:, g, g:g + 1], 1.0)

        k2 = [PG.tile([128, M], bf16, name=f"k2{g}", tag=f"k2{g}")
              for g in range(2)]
        v1 = [PG.tile([128, C], bf16, name=f"v1_{ms}", tag=f"v1_{ms}")
              for ms in range(8)]
        qTs = [[PG.tile([128, 512], bf16, name=f"qT{n}{g}", tag=f"qT{n}{g}")
                for g in range(2)] for n in range(2)]
        rstdc1 = PG.tile([128, 8], f32, name="rstdc1", tag="rstdc1")

        # ---------- phase-1 emission helpers ----------
        def conv_part(xs, base, w, oh, ck, half):
            """one half (4 matmuls) of a conv chunk for out-ch oh."""
            i0, ni = base // 32, w // 32
            if half == 0:
                ck[f"ps{oh}"] = PS.tile([128, 512], f32, name="cnv",
                                        tag="util", bufs=1)
            ps = ck[f"ps{oh}"]
            ch = half
            xv = xs[ch].rearrange("p (i ki j kj) -> p ki kj i j",
                                  ki=2, kj=2, j=32)
            k = 4 * half
            for kh in range(2):
                for kw in range(2):
                    nc.tensor.matmul(
                        ps[:, 0:w],
                        w2s[ch][:, kh, kw, oh * 128:(oh + 1) * 128],
                        xv[:, kh, kw, i0:i0 + ni, :],
                        start=(k == 0), stop=(k == 7))
                    k += 1
            if half == 1:
                xr = PG.tile([128, 512], bf16, name="xr", tag=f"xr{oh}",
                             bufs=3)
                nc.vector.tensor_scalar_add(out=xr[:, 0:w], in0=ps[:, 0:w],
                                            scalar1=srb[:, oh:oh + 1])
                ck[f"xr{oh}"] = xr

        def stats_rows(ck, w):
            xrs = [ck["xr0"], ck["xr1"]]
            sq = [PG.tile([128, 512], bf16, name="sqt", tag=f"sq{ch}", bufs=2)
                  for ch in range(2)]
            for ch in range(2):
                nc.vector.tensor_mul(sq[ch][:, 0:w], xrs[ch][:, 0:w],
                                     xrs[ch][:, 0:w])
            st = PS.tile([128, 512], f32, name="st", tag="util", bufs=1)
            for ch in range(2):
                nc.tensor.matmul(st[0:1, 0:w], onescol, xrs[ch][:, 0:w],
                                 start=(ch == 0), stop=(ch == 1),
                                 tile_position=(0, 0), skip_group_check=True)
                nc.tensor.matmul(st[32:33, 0:w], onescol, sq[ch][:, 0:w],
                                 start=(ch == 0), stop=(ch == 1),
                                 tile_position=(0, 32), skip_group_check=True)
            mnneg = PG.tile([1, 512], bf16, name="mnneg", tag="mnneg", bufs=3)
            nc.vector.tensor_scalar_mul(out=mnneg[:, 0:w], in0=st[0:1, 0:w],
                                        scalar1=-1.0 / C)
            psqs = PG.tile([1, 512], f32, name="psqs", tag="psqs", bufs=2)
            nc.vector.tensor_scalar(out=psqs[:, 0:w], in0=st[32:33, 0:w],
                                    scalar1=1.0 / C, scalar2=EPS,
                                    op0=OP.mult, op1=OP.add)
            msq = PG.tile([1, 512], f32, name="msq", tag="msq", bufs=2)
            nc.vector.tensor_mul(msq[:, 0:w], mnneg[:, 0:w], mnneg[:, 0:w])
            var = PG.tile([1, 512], f32, name="var", tag="var", bufs=2)
            nc.vector.tensor_sub(var[:, 0:w], psqs[:, 0:w], msq[:, 0:w])
            ck["mnneg"] = mnneg
            ck["var"] = var

        def newton_rows(ck, w):
            """x2 (k-side): Newton rsqrt on the var ROW -> bf16 rstd row,
            broadcast via rank-1 matmul -> rbs [128, w] bf16 SBUF."""
            var = ck["var"]
            r = PG.tile([1, 512], f32, name="rw", tag="rw", bufs=2)
            nc.vector.reciprocal_approx_fast(out=r[:, 0:w], in_=var[:, 0:w])
            x = PG.tile([1, 512], f32, name="xw", tag="xw", bufs=2)
            nc.vector.tensor_scalar(out=x[:, 0:w], in0=r[:, 0:w],
                                    scalar1=0.537, scalar2=0.340,
                                    op0=OP.mult, op1=OP.add)
            s = PG.tile([1, 512], f32, name="sw", tag="sw", bufs=2)
            u = PG.tile([1, 512], f32, name="uw", tag="uw", bufs=2)
            for it in range(2):
                nc.vector.tensor_mul(s[:, 0:w], x[:, 0:w], x[:, 0:w])
                nc.vector.tensor_mul(s[:, 0:w], s[:, 0:w], var[:, 0:w])
                nc.vector.tensor_scalar(out=u[:, 0:w], in0=s[:, 0:w],
                                        scalar1=-0.5, scalar2=1.5,
                                        op0=OP.mult, op1=OP.add)
                if it == 0:
                    nc.vector.tensor_mul(x[:, 0:w], x[:, 0:w], u[:, 0:w])
            xb = PG.tile([1, 512], bf16, name="xb", tag="xb", bufs=2)
            nc.vector.tensor_mul(xb[:, 0:w], x[:, 0:w], u[:, 0:w])
            rb = PS.tile([128, 512], f32, name="rbp", tag="util", bufs=1)
            nc.tensor.matmul(rb[:, 0:w], ones1, xb[:, 0:w],
                             start=True, stop=True)
            rbs = PG.tile([128, 512], bf16, name="rbs", tag="rbs", bufs=2)
            nc.vector.tensor_copy(out=rbs[:, 0:w], in_=rb[:, 0:w])
            ck["rbs"] = rbs

        def newton_cols(ck, base, w):
            """x1 (v-side): var row -> psum columns -> Newton -> rstdc1."""
            var = ck["var"]
            nq = w // 128
            vc = PS.tile([128, 512], f32, name="vc", tag="util", bufs=1)
            for q in range(nq):
                nc.tensor.matmul(vc[:, q:q + 1],
                                 var[:, q * 128:(q + 1) * 128], one11,
                                 start=True, stop=True,
                                 skip_group_check=True)
            r = PG.tile([128, 8], f32, name="rr", tag="rr", bufs=2)
            nc.vector.reciprocal_approx_fast(out=r[:, 0:nq], in_=vc[:, 0:nq])
            x = rstdc1[:, base // 128:base // 128 + nq]
            nc.vector.tensor_scalar(out=x, in0=r[:, 0:nq],
                                    scalar1=0.537, scalar2=0.340,
                                    op0=OP.mult, op1=OP.add)
            s = PG.tile([128, 8], f32, name="ss", tag="ss", bufs=2)
            u = PG.tile([128, 8], f32, name="uu", tag="uu", bufs=2)
            for _ in range(2):
                nc.vector.tensor_mul(s[:, 0:nq], x, x)
                nc.vector.tensor_mul(s[:, 0:nq], s[:, 0:nq], vc[:, 0:nq])
                nc.vector.tensor_scalar(out=u[:, 0:nq], in0=s[:, 0:nq],
                                        scalar1=-0.5, scalar2=1.5,
                                        op0=OP.mult, op1=OP.add)
                nc.vector.tensor_mul(x, x, u[:, 0:nq])

        def k2_half(ck, base, w, g):
            ps = PS.tile([128, 512], f32, name="k2p", tag="util", bufs=1)
            for ch in range(2):
                nc.tensor.matmul(
                    ps[:, 0:w], wk[ch][:, g * 128:(g + 1) * 128],
                    ck[f"xr{ch}"][:, 0:w], start=(ch == 0), stop=False)
            nc.tensor.matmul(ps[:, 0:w], bks[:, g * 128:(g + 1) * 128],
                             ck["mnneg"][:, 0:w], start=False, stop=True)
            nc.vector.tensor_mul(k2[g][:, base:base + w], ps[:, 0:w],
                                 ck["rbs"][:, 0:w])

        def v1_one(ck, cbase, ms):
            off = ms * 128 - cbase
            ps = PS.tile([128, 512], f32, name="v1p", tag="util", bufs=1)
            for ch in range(2):
                nc.tensor.matmul(
                    ps[:, 0:C], ck[f"xr{ch}"][:, off:off + 128], wv[ch],
                    start=(ch == 0), stop=False)
            nc.tensor.matmul(ps[:, 0:C], ck["mnneg"][:, off:off + 128], bvs,
                             start=False, stop=True)
            nc.vector.tensor_scalar_mul(out=v1[ms], in0=ps[:, 0:C],
                                        scalar1=rstdc1[:, ms:ms + 1])

        def qt_chunk(n):
            for g in range(2):
                ps = PS.tile([128, 512], f32, name="qtp", tag="util", bufs=1)
                for ch in range(2):
                    nc.tensor.matmul(
                        ps, wq[ch][:, g * 128:(g + 1) * 128],
                        xq[ch][:, n * 512:(n + 1) * 512],
                        start=(ch == 0), stop=(ch == 1))
                nc.vector.tensor_copy(out=qTs[n][g], in_=ps)

        # ---- work item list (each ~<=4 matmuls) ----
        def x2_items(ci):
            base, w = X2CH[ci]
            ck = {}
            its = [lambda oh=oh, hf=hf: conv_part(x2s, base, w, oh, ck, hf)
                   for oh in range(2) for hf in range(2)]
            its.append(lambda: (stats_rows(ck, w), newton_rows(ck, w)))
            its.append(lambda: k2_half(ck, base, w, 0))
            its.append(lambda: k2_half(ck, base, w, 1))
            return its

        def x1_items(mh):
            base, w = X1CH[mh]
            ck = {}
            its = [lambda oh=oh, hf=hf: conv_part(x1s, base, w, oh, ck, hf)
                   for oh in range(2) for hf in range(2)]
            its.append(lambda: (stats_rows(ck, w), newton_cols(ck, base, w)))
            its.append(lambda: (v1_one(ck, base, base // 128),
                                v1_one(ck, base, base // 128 + 1)))
            its.append(lambda: (v1_one(ck, base, base // 128 + 2),
                                v1_one(ck, base, base // 128 + 3)))
            return its

        work = (x2_items(1) + x1_items(0) + x2_items(2) + x1_items(1)
                + [lambda: qt_chunk(1)])
        widx = [0]

        def drain_one():
            if widx[0] < len(work):
                work[widx[0]]()
                widx[0] += 1

        # ---------------- prologue ----------------
        qt_chunk(0)
        for it in x2_items(0):
            it()

        # ---------------- attention ----------------
        for n2 in range(2):
            U = [PS.tile([128, 512], f32, name=f"U{g}", tag=f"U{g}", bufs=1)
                 for g in range(2)]
            pden = PS.tile([128, 512], f32, name="pden", tag="pden", bufs=1)

            def emit_ud(E, ms, grp, pr):
                for i in range(2):
                    j = 2 * pr + i
                    h = grp * 4 + j
                    nc.tensor.matmul(
                        U[grp][32 * j:32 * j + 32, :],
                        v1[ms][:, 32 * h:32 * h + 32],
                        E[:, i * 512:(i + 1) * 512],
                        start=(ms == 0), stop=(ms == 7),
                        tile_position=(0, 32 * j),
                        skip_group_check=True)
                for i in range(2):
                    j = 2 * pr + i
                    cpos = 32 * ((j + 2) % 4)
                    nc.tensor.matmul(
                        pden[cpos:cpos + 32, :],
                        selden[:, grp, :],
                        E[:, i * 512:(i + 1) * 512],
                        start=(ms == 0 and grp == 0),
                        stop=(ms == 7 and grp == 1),
                        tile_position=(0, cpos),
                        skip_group_check=True)

            lag = 14 if n2 == 0 else 0
            pending = []
            for blkid, (ms, grp) in enumerate(
                    (ms, grp) for ms in range(8) for grp in range(2)):
                Ss = []
                for pr in range(2):
                    S = PS.tile([128, 1024], f32, name="S", tag="scps",
                                bufs=2)
                    for i in range(2):
                        j = 2 * pr + i
                        nc.tensor.matmul(
                            S[:, i * 512:(i + 1) * 512],
                            k2[grp][32 * j:32 * j + 32,
                                    ms * 128:(ms + 1) * 128],
                            qTs[n2][grp][32 * j:32 * j + 32, :],
                            start=True, stop=True,
                            tile_position=(32 * j, 0))
                    Ss.append(S)
                for pr in range(2):
                    E = PG.tile([128, 1024], bf16, name="E", tag="E",
                                bufs=16)
                    nc.scalar.activation(out=E, in_=Ss[pr], func=AF.Exp)
                    pending.append((E, ms, grp, pr))
                if n2 == 0:
                    drain_one()
                while len(pending) > lag:
                    emit_ud(*pending.pop(0))
                if n2 == 0:
                    drain_one()
            while pending:
                emit_ud(*pending.pop(0))

            # ---- normalize + proj ----
            pdenS = PG.tile([128, 512], bf16, name="pdenS", tag="pdenS",
                            bufs=2)
            nc.vector.tensor_copy(out=pdenS, in_=pden)
            ot = []
            for g in range(2):
                rps = PS.tile([128, 512], f32, name="rps", tag="util", bufs=1)
                nc.tensor.matmul(rps, blk[g], pdenS, start=True, stop=True)
                recf = PG.tile([128, 512], f32, name="recf", tag="recf",
                               bufs=2)
                nc.vector.reciprocal_approx_fast(out=recf, in_=rps)
                o = PG.tile([128, 512], bf16, name="ot", tag=f"ot{g}", bufs=2)
                nc.vector.tensor_mul(o, U[g], recf)
                ot.append(o)
            P = PS.tile([128, 1024], f32, name="P", tag="scps", bufs=2)
            for oh in range(2):
                psl = P[:, oh * 512:(oh + 1) * 512]
                for ch in range(2):
                    nc.tensor.matmul(
                        psl, wp[ch][:, oh * 128:(oh + 1) * 128], ot[ch],
                        start=(ch == 0), stop=False)
                nc.tensor.matmul(psl, pbr[:, oh * 128:(oh + 1) * 128],
                                 ones512, start=False, stop=True)
                y = PG.tile([128, 512], f32, name="y", tag="y", bufs=2)
                nc.vector.tensor_copy(out=y, in_=psl)
                nc.gpsimd.dma_start(
                    out=outt[oh * 128:(oh + 1) * 128,
                             n2 * 512:(n2 + 1) * 512],
                    in_=y)
    nc.finalize()
    return nc


def _get_program():
    if "nc" not in _prog_cache:
        _prog_cache["nc"] = _build_program()
    return _prog_cache["nc"]


def kernel(x1, x2, q_w, kv_w, sr_w, sr_b, ln_g, ln_b, proj_w, proj_b,
           H1=64, W1=64, H2=64, W2=64, **_):
    from concourse.bass_utils import run_bass_kernel_spmd

    f = np.float32
    x1 = np.asarray(x1, f)
    x2 = np.asarray(x2, f)
    q_w = np.asarray(q_w, f)
    kv_w = np.asarray(kv_w, f)
    sr_w = np.asarray(sr_w, f)
    sr_b = np.asarray(sr_b, f)
    ln_g = np.asarray(ln_g, f)
    ln_b = np.asarray(ln_b, f)
    proj_w = np.asarray(proj_w, f)
    proj_b = np.asarray(proj_b, f)

    import ml_dtypes
    bf = ml_dtypes.bfloat16
    qwT = np.ascontiguousarray(q_w.T * SCALE).astype(bf)
    kwT = np.ascontiguousarray(ln_g[:, None] * kv_w[:C].T).astype(bf)
    vwT = np.ascontiguousarray(ln_g[:, None] * kv_w[C:].T).astype(bf)
    bksum = (kv_w[:C] @ ln_g)
    bvsum = (kv_w[C:] @ ln_g)
    bvec_v = kv_w[C:] @ ln_b
    pwT = np.ascontiguousarray(proj_w.T).astype(bf)
    w2 = np.ascontiguousarray(sr_w.transpose(2, 3, 1, 0)).astype(bf)
    pbrow = np.ascontiguousarray(
        (proj_b + proj_w @ bvec_v)[None, :]).astype(bf)
    srb2 = np.ascontiguousarray(sr_b.reshape(2, 128).T).astype(f)
    blkdm = np.zeros((2, 128, 128), bf)
    for g in range(2):
        for i in range(128):
            j = i // 32
            src = 32 * ((j + 2) % 4) + g
            blkdm[g, src, i] = 1.0
    x1T = [np.ascontiguousarray(x1[b].T).astype(bf) for b in range(B)]
    x2T = [np.ascontiguousarray(x2[b].T).astype(bf) for b in range(B)]

    in_maps = []
    for core in range(8):
        b, chk = divmod(core, 4)
        in_maps.append({
            "x1t": x1T[b], "x2t": x2T[b],
            "xqt": np.ascontiguousarray(x1T[b][:, chk * NCH:(chk + 1) * NCH]),
            "w2": w2, "qwt": qwT, "kwt": kwT, "vwt": vwT, "pwt": pwT,
            "pbrow": pbrow, "srb2": srb2, "blkd": blkdm,
            "bksd": np.ascontiguousarray(bksum[None, :]).astype(bf),
            "bvsd": np.ascontiguousarray(bvsum[None, :]).astype(bf),
        })

    nc = _get_program()
    res = run_bass_kernel_spmd(nc, in_maps, core_ids=list(range(8)))
    out = np.empty((B, N, C), f)
    for core in range(8):
        b, chk = divmod(core, 4)
        out[b, chk * NCH:(chk + 1) * NCH, :] = res.results[core]["outt"].T
    return out


# revision 15
# speedup vs baseline: 1.4120x; 1.0813x over previous
"""CrossTemporalAttention2 Trainium2 kernel (pipelined rewrite, v3).

Sharding: 8 cores = 2 batches x 4 query-chunks of 1024 rows.
Each core: conv+LN+KV for its batch (duplicated across the 4 cores of the
batch group), attention + proj for its 1024 query rows.

v3 structure:
  - phase-1 (conv/stats/k2/v1/qT) is flattened into micro work items
    (<=4 matmuls each) drained two per attention block, so the Tensor
    engine stream stays dense (p-state!) and exp starts early.
  - LN is never materialized. k side: k2 = (kwT@xr + (-mu)@bksum) * rstd2
    where rstd2 is a row computed by a DVE Newton rsqrt and applied via a
    rank-1 broadcast matmul + tensor_tensor multiply at the PSUM->SBUF
    copy. v side: rstd1 columns (PE row->col transpose + DVE Newton)
    applied as per-partition tensor_scalar multiply. exp stays scale-free
    (a per-partition scale AP costs ~+50% on the ACT engine).
  - LN beta: k side cancels in softmax exactly; v side folds into proj
    bias on host. conv bias via per-partition tensor_scalar add.
  - attention per (ms, grp) block: 4 score matmuls row-tiled at
    (32j, 0) -> all 4 concurrent in the PE array; 2 exps [128,1024];
    U (AV) + den matmuls form 4-up col-tiled concurrent sets.
  - denominator windows are written full (32 rows) so no uninitialized
    PSUM is ever read; broadcast per head via host permutation matmul.
  - n2=0 defers U/den by `lag` exp-pairs so v1[ms] emission always
    precedes its consumers (E pool holds lag+2 tiles).

PSUM (8 banks): U0,U1,pden (3) + scores 2x[128,1024] (4) + util (1).
"""

import numpy as np

B, N, C = 2, 4096, 256
H, Dh = 8, 32
M = 1024
NCH = 1024
SCALE = Dh ** -0.5
EPS = 1e-5

X2CH = [(0, 256), (256, 512), (768, 256)]   # k-side conv chunks
X1CH = [(0, 512), (512, 512)]               # v-side conv chunks

_prog_cache = {}


def _build_program():
    import concourse.bass as bass
    import concourse.bacc as bacc
    import concourse.tile as tile
    from concourse import mybir

    f32 = mybir.dt.float32
    bf16 = mybir.dt.bfloat16
    AF = mybir.ActivationFunctionType
    OP = mybir.AluOpType

    nc = bacc.Bacc()

    x1t = nc.dram_tensor("x1t", [C, N], bf16, kind="ExternalInput")
    x2t = nc.dram_tensor("x2t", [C, N], bf16, kind="ExternalInput")
    xqt = nc.dram_tensor("xqt", [C, NCH], bf16, kind="ExternalInput")
    w2d = nc.dram_tensor("w2", [2, 2, C, C], bf16, kind="ExternalInput")
    qwt = nc.dram_tensor("qwt", [C, C], bf16, kind="ExternalInput")
    kwt = nc.dram_tensor("kwt", [C, C], bf16, kind="ExternalInput")
    vwt = nc.dram_tensor("vwt", [C, C], bf16, kind="ExternalInput")
    pwt = nc.dram_tensor("pwt", [C, C], bf16, kind="ExternalInput")
    pbrow = nc.dram_tensor("pbrow", [1, C], bf16, kind="ExternalInput")
    bksd = nc.dram_tensor("bksd", [1, C], bf16, kind="ExternalInput")
    bvsd = nc.dram_tensor("bvsd", [1, C], bf16, kind="ExternalInput")
    srb2 = nc.dram_tensor("srb2", [128, 2], f32, kind="ExternalInput")
    blkd = nc.dram_tensor("blkd", [2, 128, 128], bf16, kind="ExternalInput")
    outt = nc.dram_tensor("outt", [C, NCH], f32, kind="ExternalOutput")

    with nc.allow_low_precision(reason="bf16 matmul inputs; fp32 PSUM accum"), \
         tile.TileContext(nc) as tc:
      with tc.tile_pool(name="pg", bufs=1) as PG, \
           tc.tile_pool(name="psum", bufs=1, space="PSUM") as PS:

        w2s = [PG.tile([128, 2, 2, C], bf16, name=f"w2{ch}", tag=f"w2{ch}")
               for ch in range(2)]
        wq = [PG.tile([128, C], bf16, name=f"wq{ch}", tag=f"wq{ch}")
              for ch in range(2)]
        wk = [PG.tile([128, C], bf16, name=f"wk{ch}", tag=f"wk{ch}")
              for ch in range(2)]
        wv = [PG.tile([128, C], bf16, name=f"wv{ch}", tag=f"wv{ch}")
              for ch in range(2)]
        wp = [PG.tile([128, C], bf16, name=f"wp{ch}", tag=f"wp{ch}")
              for ch in range(2)]
        x2s = [PG.tile([128, N], bf16, name=f"x2s{ch}", tag=f"x2s{ch}")
               for ch in range(2)]
        x1s = [PG.tile([128, N], bf16, name=f"x1s{ch}", tag=f"x1s{ch}")
               for ch in range(2)]
        xq = [PG.tile([128, NCH], bf16, name=f"xq{ch}", tag=f"xq{ch}")
              for ch in range(2)]
        pbr = PG.tile([1, C], bf16, name="pbr", tag="pbr")
        bks = PG.tile([1, C], bf16, name="bks", tag="bks")
        bvs = PG.tile([1, C], bf16, name="bvs", tag="bvs")
        srb = PG.tile([128, 2], f32, name="srb", tag="srb")
        blk = [PG.tile([128, 128], bf16, name=f"blk{g}", tag=f"blk{g}")
               for g in range(2)]

        # DMA priority order: conv-c0 critical path first.
        nc.gpsimd.dma_start(out=w2s[0], in_=w2d[:, :, 0:128, :].rearrange(
            "kh kw c o -> c kh kw o"))
        nc.gpsimd.dma_start(out=w2s[1], in_=w2d[:, :, 128:256, :].rearrange(
            "kh kw c o -> c kh kw o"))
        for ch in range(2):   # conv input for x2 chunk c0 ([0:256) -> cols 0:1024)
            nc.gpsimd.dma_start(out=x2s[ch][:, 0:1024],
                                in_=x2t[ch * 128:(ch + 1) * 128, 0:1024])
        nc.gpsimd.dma_start(out=srb, in_=srb2[:, :])
        nc.gpsimd.dma_start(out=bks, in_=bksd[:, :])
        for ch in range(2):
            nc.gpsimd.dma_start(out=wk[ch], in_=kwt[ch * 128:(ch + 1) * 128, :])
        for ch in range(2):
            nc.gpsimd.dma_start(out=wq[ch], in_=qwt[ch * 128:(ch + 1) * 128, :])
        for ch in range(2):
            nc.gpsimd.dma_start(out=xq[ch], in_=xqt[ch * 128:(ch + 1) * 128, :])
        for ch in range(2):   # x2 chunk c1 ([256:768) -> cols 1024:3072)
            nc.gpsimd.dma_start(out=x2s[ch][:, 1024:3072],
                                in_=x2t[ch * 128:(ch + 1) * 128, 1024:3072])
        for ch in range(2):   # x1 chunk mh0 (cols 0:2048)
            nc.gpsimd.dma_start(out=x1s[ch][:, 0:2048],
                                in_=x1t[ch * 128:(ch + 1) * 128, 0:2048])
        for ch in range(2):
            nc.gpsimd.dma_start(out=wv[ch], in_=vwt[ch * 128:(ch + 1) * 128, :])
        nc.gpsimd.dma_start(out=bvs, in_=bvsd[:, :])
        for ch in range(2):   # x2 chunk c2 (cols 3072:4096)
            nc.gpsimd.dma_start(out=x2s[ch][:, 3072:4096],
                                in_=x2t[ch * 128:(ch + 1) * 128, 3072:4096])
        for ch in range(2):   # x1 chunk mh1 (cols 2048:4096)
            nc.gpsimd.dma_start(out=x1s[ch][:, 2048:4096],
                                in_=x1t[ch * 128:(ch + 1) * 128, 2048:4096])
        for ch in range(2):
            nc.gpsimd.dma_start(out=wp[ch], in_=pwt[ch * 128:(ch + 1) * 128, :])
        nc.gpsimd.dma_start(out=pbr, in_=pbrow[:, :])
        for g in range(2):
            nc.gpsimd.dma_start(out=blk[g], in_=blkd[g])

        onescol = PG.tile([128, 1], bf16, name="onescol", tag="onescol")
        nc.vector.memset(onescol, 1.0)
        one11 = PG.tile([1, 1], f32, name="one11", tag="one11")
        nc.vector.memset(one11, 1.0)
        ones1 = PG.tile([1, 128], bf16, name="ones1", tag="ones1")
        nc.vector.memset(ones1, 1.0)
        ones512 = PG.tile([1, 512], bf16, name="ones512", tag="ones512")
        nc.vector.memset(ones512, 1.0)
        selden = PG.tile([128, 2, 32], bf16, name="selden", tag="selden")
        nc.vector.memset(selden, 0.0)
        for g in range(2):
            nc.vector.memset(selden[:, g, g:g + 1], 1.0)

        k2 = [PG.tile([128, M], bf16, name=f"k2{g}", tag=f"k2{g}")
              for g in range(2)]
        v1 = [PG.tile([128, C], bf16, name=f"v1_{ms}", tag=f"v1_{ms}")
              for ms in range(8)]
        qTs = [[PG.tile([128, 512], bf16, name=f"qT{n}{g}", tag=f"qT{n}{g}")
                for g in range(2)] for n in range(2)]
        rstdc1 = PG.tile([128, 8], f32, name="rstdc1", tag="rstdc1")

        # ---------- phase-1 emission helpers ----------
        def conv_part(xs, base, w, oh, ck, half):
            """one half (4 matmuls) of a conv chunk for out-ch oh."""
            i0, ni = base // 32, w // 32
            if half == 0:
                ck[f"ps{oh}"] = PS.tile([128, 512], f32, name="cnv",
                                        tag="util", bufs=1)
            ps = ck[f"ps{oh}"]
            ch = half
            xv = xs[ch].rearrange("p (i ki j kj) -> p ki kj i j",
                                  ki=2, kj=2, j=32)
            k = 4 * half
            for kh in range(2):
                for kw in range(2):
                    nc.tensor.matmul(
                        ps[:, 0:w],
                        w2s[ch][:, kh, kw, oh * 128:(oh + 1) * 128],
                        xv[:, kh, kw, i0:i0 + ni, :],
                        start=(k == 0), stop=(k == 7))
                    k += 1
            if half == 1:
                xr = PG.tile([128, 512], bf16, name="xr", tag=f"xr{oh}",
                             bufs=3)
                nc.vector.tensor_scalar_add(out=xr[:, 0:w], in0=ps[:, 0:w],
                                            scalar1=srb[:, oh:oh + 1])
                ck[f"xr{oh}"] = xr

        def stats_rows(ck, w):
            xrs = [ck["xr0"], ck["xr1"]]
            sq = [PG.tile([128, 512], bf16, name="sqt", tag=f"sq{ch}", bufs=2)
                  for ch in range(2)]
            for ch in range(2):
                nc.vector.tensor_mul(sq[ch][:, 0:w], xrs[ch][:, 0:w],
                                     xrs[ch][:, 0:w])
            st = PS.tile([128, 512], f32, name="st", tag="util", bufs=1)
            for ch in range(2):
                nc.tensor.matmul(st[0:1, 0:w], onescol, xrs[ch][:, 0:w],
                                 start=(ch == 0), stop=(ch == 1),
                                 tile_position=(0, 0), skip_group_check=True)
                nc.tensor.matmul(st[32:33, 0:w], onescol, sq[ch][:, 0:w],
                                 start=(ch == 0), stop=(ch == 1),
                                 tile_position=(0, 32), skip_group_check=True)
            mnneg = PG.tile([1, 512], bf16, name="mnneg", tag="mnneg", bufs=3)
            nc.vector.tensor_scalar_mul(out=mnneg[:, 0:w], in0=st[0:1, 0:w],
                                        scalar1=-1.0 / C)
            psqs = PG.tile([1, 512], f32, name="psqs", tag="psqs", bufs=2)
            nc.vector.tensor_scalar(out=psqs[:, 0:w], in0=st[32:33, 0:w],
                                    scalar1=1.0 / C, scalar2=EPS,
                                    op0=OP.mult, op1=OP.add)
            msq = PG.tile([1, 512], f32, name="msq", tag="msq", bufs=2)
            nc.vector.tensor_mul(msq[:, 0:w], mnneg[:, 0:w], mnneg[:, 0:w])
            var = PG.tile([1, 512], f32, name="var", tag="var", bufs=2)
            nc.vector.tensor_sub(var[:, 0:w], psqs[:, 0:w], msq[:, 0:w])
            ck["mnneg"] = mnneg
            ck["var"] = var

        def newton_rows(ck, w):
            """x2 (k-side): Newton rsqrt on the var ROW -> bf16 rstd row,
            broadcast via rank-1 matmul -> rbs [128, w] bf16 SBUF."""
            var = ck["var"]
            r = PG.tile([1, 512], f32, name="rw", tag="rw", bufs=2)
            nc.vector.reciprocal_approx_fast(out=r[:, 0:w], in_=var[:, 0:w])
            x = PG.tile([1, 512], f32, name="xw", tag="xw", bufs=2)
            nc.vector.tensor_scalar(out=x[:, 0:w], in0=r[:, 0:w],
                                    scalar1=0.537, scalar2=0.340,
                                    op0=OP.mult, op1=OP.add)
            s = PG.tile([1, 512], f32, name="sw", tag="sw", bufs=2)
            u = PG.tile([1, 512], f32, name="uw", tag="uw", bufs=2)
            for it in range(2):
                nc.vector.tensor_mul(s[:, 0:w], x[:, 0:w], x[:, 0:w])
                nc.vector.tensor_mul(s[:, 0:w], s[:, 0:w], var[:, 0:w])
                nc.vector.tensor_scalar(out=u[:, 0:w], in0=s[:, 0:w],
                                        scalar1=-0.5, scalar2=1.5,
                                        op0=OP.mult, op1=OP.add)
                if it == 0:
                    nc.vector.tensor_mul(x[:, 0:w], x[:, 0:w], u[:, 0:w])
            xb = PG.tile([1, 512], bf16, name="xb", tag="xb", bufs=2)
            nc.vector.tensor_mul(xb[:, 0:w], x[:, 0:w], u[:, 0:w])
            rb = PS.tile([128, 512], f32, name="rbp", tag="util", bufs=1)
            nc.tensor.matmul(rb[:, 0:w], ones1, xb[:, 0:w],
                             start=True, stop=True)
            rbs = PG.tile([128, 512], bf16, name="rbs", tag="rbs", bufs=2)
            nc.vector.tensor_copy(out=rbs[:, 0:w], in_=rb[:, 0:w])
            ck["rbs"] = rbs

        def newton_cols(ck, base, w):
            """x1 (v-side): var row -> psum columns -> Newton -> rstdc1."""
            var = ck["var"]
            nq = w // 128
            vc = PS.tile([128, 512], f32, name="vc", tag="util", bufs=1)
            for q in range(nq):
                nc.tensor.matmul(vc[:, q:q + 1],
                                 var[:, q * 128:(q + 1) * 128], one11,
                                 start=True, stop=True,
                                 skip_group_check=True)
            r = PG.tile([128, 8], f32, name="rr", tag="rr", bufs=2)
            nc.vector.reciprocal_approx_fast(out=r[:, 0:nq], in_=vc[:, 0:nq])
            x = rstdc1[:, base // 128:base // 128 + nq]
            nc.vector.tensor_scalar(out=x, in0=r[:, 0:nq],
                                    scalar1=0.537, scalar2=0.340,
                                    op0=OP.mult, op1=OP.add)
            s = PG.tile([128, 8], f32, name="ss", tag="ss", bufs=2)
            u = PG.tile([128, 8], f32, name="uu", tag="uu", bufs=2)
            for _ in range(2):
                nc.vector.tensor_mul(s[:, 0:nq], x, x)
                nc.vector.tensor_mul(s[:, 0:nq], s[:, 0:nq], vc[:, 0:nq])
                nc.vector.tensor_scalar(out=u[:, 0:nq], in0=s[:, 0:nq],
                                        scalar1=-0.5, scalar2=1.5,
                                        op0=OP.mult, op1=OP.add)
                nc.vector.tensor_mul(x, x, u[:, 0:nq])

        def k2_half(ck, base, w, g):
            ps = PS.tile([128, 512], f32, name="k2p", tag="util", bufs=1)
            for ch in range(2):
                nc.tensor.matmul(
                    ps[:, 0:w], wk[ch][:, g * 128:(g + 1) * 128],
                    ck[f"xr{ch}"][:, 0:w], start=(ch == 0), stop=False)
            nc.tensor.matmul(ps[:, 0:w], bks[:, g * 128:(g + 1) * 128],
                             ck["mnneg"][:, 0:w], start=False, stop=True)
            nc.vector.tensor_mul(k2[g][:, base:base + w], ps[:, 0:w],
                                 ck["rbs"][:, 0:w])

        def v1_one(ck, cbase, ms):
            off = ms * 128 - cbase
            ps = PS.tile([128, 512], f32, name="v1p", tag="util", bufs=1)
            for ch in range(2):
                nc.tensor.matmul(
                    ps[:, 0:C], ck[f"xr{ch}"][:, off:off + 128], wv[ch],
                    start=(ch == 0), stop=False)
            nc.tensor.matmul(ps[:, 0:C], ck["mnneg"][:, off:off + 128], bvs,
                             start=False, stop=True)
            nc.vector.tensor_scalar_mul(out=v1[ms], in0=ps[:, 0:C],
                                        scalar1=rstdc1[:, ms:ms + 1])

        def qt_chunk(n):
            for g in range(2):
                ps = PS.tile([128, 512], f32, name="qtp", tag="util", bufs=1)
                for ch in range(2):
                    nc.tensor.matmul(
                        ps, wq[ch][:, g * 128:(g + 1) * 128],
                        xq[ch][:, n * 512:(n + 1) * 512],
                        start=(ch == 0), stop=(ch == 1))
                nc.vector.tensor_copy(out=qTs[n][g], in_=ps)

        # ---- work item list (each ~<=4 matmuls) ----
        def x2_items(ci):
            base, w = X2CH[ci]
            ck = {}
            its = [lambda oh=oh, hf=hf: conv_part(x2s, base, w, oh, ck, hf)
                   for oh in range(2) for hf in range(2)]
            its.append(lambda: (stats_rows(ck, w), newton_rows(ck, w)))
            its.append(lambda: k2_half(ck, base, w, 0))
            its.append(lambda: k2_half(ck, base, w, 1))
            return its

        def x1_items(mh):
            base, w = X1CH[mh]
            ck = {}
            its = [lambda oh=oh, hf=hf: conv_part(x1s, base, w, oh, ck, hf)
                   for oh in range(2) for hf in range(2)]
            its.append(lambda: (stats_rows(ck, w), newton_cols(ck, base, w)))
            its.append(lambda: (v1_one(ck, base, base // 128),
                                v1_one(ck, base, base // 128 + 1)))
            its.append(lambda: (v1_one(ck, base, base // 128 + 2),
                                v1_one(ck, base, base // 128 + 3)))
            return its

        x1a = x1_items(0)
        x1b = x1_items(1)
        x2c2 = x2_items(2)
        work = (x2_items(1) + x1a[:4] + x2c2[:4] + x1a[4:]
                + x2c2[4:] + x1b + [lambda: qt_chunk(1)])
        widx = [0]

        def drain_one():
            if widx[0] < len(work):
                work[widx[0]]()
                widx[0] += 1

        # ---------------- prologue ----------------
        qt_chunk(0)
        for it in x2_items(0):
            it()

        # ---------------- attention ----------------
        for n2 in range(2):
            U = [PS.tile([128, 512], f32, name=f"U{g}", tag=f"U{g}", bufs=1)
                 for g in range(2)]
            pden = PS.tile([128, 512], f32, name="pden", tag="pden", bufs=1)

            def emit_ud(E, ms, grp, pr):
                for i in range(2):
                    j = 2 * pr + i
                    h = grp * 4 + j
                    nc.tensor.matmul(
                        U[grp][32 * j:32 * j + 32, :],
                        v1[ms][:, 32 * h:32 * h + 32],
                        E[:, i * 512:(i + 1) * 512],
                        start=(ms == 0), stop=(ms == 7),
                        tile_position=(0, 32 * j),
                        skip_group_check=True)
                for i in range(2):
                    j = 2 * pr + i
                    cpos = 32 * ((j + 2) % 4)
                    nc.tensor.matmul(
                        pden[cpos:cpos + 32, :],
                        selden[:, grp, :],
                        E[:, i * 512:(i + 1) * 512],
                        start=(ms == 0 and grp == 0),
                        stop=(ms == 7 and grp == 1),
                        tile_position=(0, cpos),
                        skip_group_check=True)

            pending = []
            for blkid, (ms, grp) in enumerate(
                    (ms, grp) for ms in range(8) for grp in range(2)):
                if n2 == 0:
                    # drain all phase-1 work over the first 10 blocks, then
                    # taper the U/den lag so there is no exp-free flush tail
                    lag = 14 if blkid <= 8 else max(0, 14 - 2 * (blkid - 8))
                else:
                    lag = 4
                Ss = []
                for pr in range(2):
                    S = PS.tile([128, 1024], f32, name="S", tag="scps",
                                bufs=2)
                    for i in range(2):
                        j = 2 * pr + i
                        nc.tensor.matmul(
                            S[:, i * 512:(i + 1) * 512],
                            k2[grp][32 * j:32 * j + 32,
                                    ms * 128:(ms + 1) * 128],
                            qTs[n2][grp][32 * j:32 * j + 32, :],
                            start=True, stop=True,
                            tile_position=(32 * j, 0))
                    Ss.append(S)
                for pr in range(2):
                    E = PG.tile([128, 1024], bf16, name="E", tag="E",
                                bufs=16)
                    nc.scalar.activation(out=E, in_=Ss[pr], func=AF.Exp)
                    pending.append((E, ms, grp, pr))
                if n2 == 0:
                    drain_one()
                while len(pending) > lag:
                    emit_ud(*pending.pop(0))
                if n2 == 0:
                    drain_one()
                    drain_one()
            while pending:
                emit_ud(*pending.pop(0))

            # ---- normalize + proj ----
            pdenS = PG.tile([128, 512], bf16, name="pdenS", tag="pdenS",
                            bufs=2)
            nc.vector.tensor_copy(out=pdenS, in_=pden)
            ot = []
            for g in range(2):
                rps = PS.tile([128, 512], f32, name="rps", tag="util", bufs=1)
                nc.tensor.matmul(rps, blk[g], pdenS, start=True, stop=True)
                recf = PG.tile([128, 512], f32, name="recf", tag="recf",
                               bufs=2)
                nc.vector.reciprocal_approx_fast(out=recf, in_=rps)
                o = PG.tile([128, 512], bf16, name="ot", tag=f"ot{g}", bufs=2)
                nc.vector.tensor_mul(o, U[g], recf)
                ot.append(o)
            P = PS.tile([128, 1024], f32, name="P", tag="scps", bufs=2)
            for oh in range(2):
                psl = P[:, oh * 512:(oh + 1) * 512]
                for ch in range(2):
                    nc.tensor.matmul(
                        psl, wp[ch][:, oh * 128:(oh + 1) * 128], ot[ch],
                        start=(ch == 0), stop=False)
                nc.tensor.matmul(psl, pbr[:, oh * 128:(oh + 1) * 128],
                                 ones512, start=False, stop=True)
                y = PG.tile([128, 512], f32, name="y", tag="y", bufs=2)
                nc.vector.tensor_copy(out=y, in_=psl)
                nc.gpsimd.dma_start(
                    out=outt[oh * 128:(oh + 1) * 128,
                             n2 * 512:(n2 + 1) * 512],
                    in_=y)
    nc.finalize()
    return nc


def _get_program():
    if "nc" not in _prog_cache:
        _prog_cache["nc"] = _build_program()
    return _prog_cache["nc"]


def kernel(x1, x2, q_w, kv_w, sr_w, sr_b, ln_g, ln_b, proj_w, proj_b,
           H1=64, W1=64, H2=64, W2=64, **_):
    from concourse.bass_utils import run_bass_kernel_spmd

    f = np.float32
    x1 = np.asarray(x1, f)
    x2 = np.asarray(x2, f)
    q_w = np.asarray(q_w, f)
    kv_w = np.asarray(kv_w, f)
    sr_w = np.asarray(sr_w, f)
    sr_b = np.asarray(sr_b, f)
    ln_g = np.asarray(ln_g, f)
    ln_b = np.asarray(ln_b, f)
    proj_w = np.asarray(proj_w, f)
    proj_b = np.asarray(proj_b, f)

    import ml_dtypes
    bf = ml_dtypes.bfloat16
    qwT = np.ascontiguousarray(q_w.T * SCALE).astype(bf)
    kwT = np.ascontiguousarray(ln_g[:, None] * kv_w[:C].T).astype(bf)
    vwT = np.ascontiguousarray(ln_g[:, None] * kv_w[C:].T).astype(bf)
    bksum = (kv_w[:C] @ ln_g)
    bvsum = (kv_w[C:] @ ln_g)
    bvec_v = kv_w[C:] @ ln_b
    pwT = np.ascontiguousarray(proj_w.T).astype(bf)
    w2 = np.ascontiguousarray(sr_w.transpose(2, 3, 1, 0)).astype(bf)
    pbrow = np.ascontiguousarray(
        (proj_b + proj_w @ bvec_v)[None, :]).astype(bf)
    srb2 = np.ascontiguousarray(sr_b.reshape(2, 128).T).astype(f)
    blkdm = np.zeros((2, 128, 128), bf)
    for g in range(2):
        for i in range(128):
            j = i // 32
            src = 32 * ((j + 2) % 4) + g
            blkdm[g, src, i] = 1.0
    x1T = [np.ascontiguousarray(x1[b].T).astype(bf) for b in range(B)]
    x2T = [np.ascontiguousarray(x2[b].T).astype(bf) for b in range(B)]

    in_maps = []
    for core in range(8):
        b, chk = divmod(core, 4)
        in_maps.append({
            "x1t": x1T[b], "x2t": x2T[b],
            "xqt": np.ascontiguousarray(x1T[b][:, chk * NCH:(chk + 1) * NCH]),
            "w2": w2, "qwt": qwT, "kwt": kwT, "vwt": vwT, "pwt": pwT,
            "pbrow": pbrow, "srb2": srb2, "blkd": blkdm,
            "bksd": np.ascontiguousarray(bksum[None, :]).astype(bf),
            "bvsd": np.ascontiguousarray(bvsum[None, :]).astype(bf),
        })

    nc = _get_program()
    res = run_bass_kernel_spmd(nc, in_maps, core_ids=list(range(8)))
    out = np.empty((B, N, C), f)
    for core in range(8):
        b, chk = divmod(core, 4)
        out[b, chk * NCH:(chk + 1) * NCH, :] = res.results[core]["outt"].T
    return out


# revision 19
# speedup vs baseline: 1.5150x; 1.0730x over previous
"""CrossTemporalAttention2 Trainium2 kernel (pipelined rewrite, v3).

Sharding: 8 cores = 2 batches x 4 query-chunks of 1024 rows.
Each core: conv+LN+KV for its batch (duplicated across the 4 cores of the
batch group), attention + proj for its 1024 query rows.

v3 structure:
  - phase-1 (conv/stats/k2/v1/qT) is flattened into micro work items
    (<=4 matmuls each) drained two per attention block, so the Tensor
    engine stream stays dense (p-state!) and exp starts early.
  - LN is never materialized. k side: k2 = (kwT@xr + (-mu)@bksum) * rstd2
    where rstd2 is a row computed by a DVE Newton rsqrt and applied via a
    rank-1 broadcast matmul + tensor_tensor multiply at the PSUM->SBUF
    copy. v side: rstd1 columns (PE row->col transpose + DVE Newton)
    applied as per-partition tensor_scalar multiply. exp stays scale-free
    (a per-partition scale AP costs ~+50% on the ACT engine).
  - LN beta: k side cancels in softmax exactly; v side folds into proj
    bias on host. conv bias via per-partition tensor_scalar add.
  - attention per (ms, grp) block: 4 score matmuls row-tiled at
    (32j, 0) -> all 4 concurrent in the PE array; 2 exps [128,1024];
    U (AV) + den matmuls form 4-up col-tiled concurrent sets.
  - denominator windows are written full (32 rows) so no uninitialized
    PSUM is ever read; broadcast per head via host permutation matmul.
  - n2=0 defers U/den by `lag` exp-pairs so v1[ms] emission always
    precedes its consumers (E pool holds lag+2 tiles).

PSUM (8 banks): U0,U1,pden (3) + scores 2x[128,1024] (4) + util (1).
"""

import numpy as np

B, N, C = 2, 4096, 256
H, Dh = 8, 32
M = 1024
NCH = 1024
SCALE = Dh ** -0.5
EPS = 1e-5

X2CH = [(0, 256), (256, 512), (768, 256)]   # k-side conv chunks
X1CH = [(0, 512), (512, 512)]               # v-side conv chunks

_prog_cache = {}


def _build_program():
    import concourse.bass as bass
    import concourse.bacc as bacc
    import concourse.tile as tile
    from concourse import mybir

    f32 = mybir.dt.float32
    bf16 = mybir.dt.bfloat16
    AF = mybir.ActivationFunctionType
    OP = mybir.AluOpType

    nc = bacc.Bacc()

    x1t = nc.dram_tensor("x1t", [C, N], bf16, kind="ExternalInput")
    x2t = nc.dram_tensor("x2t", [C, N], bf16, kind="ExternalInput")
    xqt = nc.dram_tensor("xqt", [C, NCH], bf16, kind="ExternalInput")
    w2d = nc.dram_tensor("w2", [2, 2, C, C], bf16, kind="ExternalInput")
    qwt = nc.dram_tensor("qwt", [C, C], bf16, kind="ExternalInput")
    kwt = nc.dram_tensor("kwt", [C, C], bf16, kind="ExternalInput")
    vwt = nc.dram_tensor("vwt", [C, C], bf16, kind="ExternalInput")
    pwt = nc.dram_tensor("pwt", [C, C], bf16, kind="ExternalInput")
    pbrow = nc.dram_tensor("pbrow", [1, C], bf16, kind="ExternalInput")
    bksd = nc.dram_tensor("bksd", [1, C], bf16, kind="ExternalInput")
    bvsd = nc.dram_tensor("bvsd", [1, C], bf16, kind="ExternalInput")
    srb2 = nc.dram_tensor("srb2", [128, 2], f32, kind="ExternalInput")
    blkd = nc.dram_tensor("blkd", [2, 128, 128], bf16, kind="ExternalInput")
    outt = nc.dram_tensor("outt", [C, NCH], f32, kind="ExternalOutput")

    with nc.allow_low_precision(reason="bf16 matmul inputs; fp32 PSUM accum"), \
         tile.TileContext(nc) as tc:
      with tc.tile_pool(name="pg", bufs=1) as PG, \
           tc.tile_pool(name="psum", bufs=1, space="PSUM") as PS:

        w2s = [PG.tile([128, 2, 2, C], bf16, name=f"w2{ch}", tag=f"w2{ch}")
               for ch in range(2)]
        wq = [PG.tile([128, C], bf16, name=f"wq{ch}", tag=f"wq{ch}")
              for ch in range(2)]
        wk = [PG.tile([128, C], bf16, name=f"wk{ch}", tag=f"wk{ch}")
              for ch in range(2)]
        wv = [PG.tile([128, C], bf16, name=f"wv{ch}", tag=f"wv{ch}")
              for ch in range(2)]
        wp = [PG.tile([128, C], bf16, name=f"wp{ch}", tag=f"wp{ch}")
              for ch in range(2)]
        x2s = [PG.tile([128, N], bf16, name=f"x2s{ch}", tag=f"x2s{ch}")
               for ch in range(2)]
        x1s = [PG.tile([128, N], bf16, name=f"x1s{ch}", tag=f"x1s{ch}")
               for ch in range(2)]
        xq = [PG.tile([128, NCH], bf16, name=f"xq{ch}", tag=f"xq{ch}")
              for ch in range(2)]
        pbr = PG.tile([1, C], bf16, name="pbr", tag="pbr")
        bks = PG.tile([1, C], bf16, name="bks", tag="bks")
        bvs = PG.tile([1, C], bf16, name="bvs", tag="bvs")
        srb = PG.tile([128, 2], f32, name="srb", tag="srb")
        blk = [PG.tile([128, 128], bf16, name=f"blk{g}", tag=f"blk{g}")
               for g in range(2)]

        # DMA priority order: conv-c0 critical path first.
        nc.gpsimd.dma_start(out=w2s[0], in_=w2d[:, :, 0:128, :].rearrange(
            "kh kw c o -> c kh kw o"))
        nc.gpsimd.dma_start(out=w2s[1], in_=w2d[:, :, 128:256, :].rearrange(
            "kh kw c o -> c kh kw o"))
        for ch in range(2):   # conv input for x2 chunk c0 ([0:256) -> cols 0:1024)
            nc.gpsimd.dma_start(out=x2s[ch][:, 0:1024],
                                in_=x2t[ch * 128:(ch + 1) * 128, 0:1024])
        nc.gpsimd.dma_start(out=srb, in_=srb2[:, :])
        for ch in range(2):
            nc.gpsimd.dma_start(out=xq[ch], in_=xqt[ch * 128:(ch + 1) * 128, :])
        for ch in range(2):
            nc.gpsimd.dma_start(out=wq[ch], in_=qwt[ch * 128:(ch + 1) * 128, :])
        for ch in range(2):
            nc.gpsimd.dma_start(out=wk[ch], in_=kwt[ch * 128:(ch + 1) * 128, :])
        nc.gpsimd.dma_start(out=bks, in_=bksd[:, :])
        for ch in range(2):   # x2 chunk c1 ([256:768) -> cols 1024:3072)
            nc.gpsimd.dma_start(out=x2s[ch][:, 1024:3072],
                                in_=x2t[ch * 128:(ch + 1) * 128, 1024:3072])
        for ch in range(2):   # x1 chunk mh0 (cols 0:2048)
            nc.gpsimd.dma_start(out=x1s[ch][:, 0:2048],
                                in_=x1t[ch * 128:(ch + 1) * 128, 0:2048])
        for ch in range(2):
            nc.gpsimd.dma_start(out=wv[ch], in_=vwt[ch * 128:(ch + 1) * 128, :])
        nc.gpsimd.dma_start(out=bvs, in_=bvsd[:, :])
        for ch in range(2):   # x2 chunk c2 (cols 3072:4096)
            nc.gpsimd.dma_start(out=x2s[ch][:, 3072:4096],
                                in_=x2t[ch * 128:(ch + 1) * 128, 3072:4096])
        for ch in range(2):   # x1 chunk mh1 (cols 2048:4096)
            nc.gpsimd.dma_start(out=x1s[ch][:, 2048:4096],
                                in_=x1t[ch * 128:(ch + 1) * 128, 2048:4096])
        for ch in range(2):
            nc.gpsimd.dma_start(out=wp[ch], in_=pwt[ch * 128:(ch + 1) * 128, :])
        nc.gpsimd.dma_start(out=pbr, in_=pbrow[:, :])
        for g in range(2):
            nc.gpsimd.dma_start(out=blk[g], in_=blkd[g])

        onescol = PG.tile([128, 1], bf16, name="onescol", tag="onescol")
        nc.vector.memset(onescol, 1.0)
        one11 = PG.tile([1, 1], f32, name="one11", tag="one11")
        nc.vector.memset(one11, 1.0)
        ones1 = PG.tile([1, 128], bf16, name="ones1", tag="ones1")
        nc.vector.memset(ones1, 1.0)
        ones512 = PG.tile([1, 512], bf16, name="ones512", tag="ones512")
        nc.vector.memset(ones512, 1.0)
        selden = PG.tile([128, 2, 32], bf16, name="selden", tag="selden")
        nc.vector.memset(selden, 0.0)
        for g in range(2):
            nc.vector.memset(selden[:, g, g:g + 1], 1.0)

        k2 = [PG.tile([128, M], bf16, name=f"k2{g}", tag=f"k2{g}")
              for g in range(2)]
        v1 = [PG.tile([128, C], bf16, name=f"v1_{ms}", tag=f"v1_{ms}")
              for ms in range(8)]
        qTs = [[PG.tile([128, 512], bf16, name=f"qT{n}{g}", tag=f"qT{n}{g}")
                for g in range(2)] for n in range(2)]
        rstdc1 = PG.tile([128, 8], f32, name="rstdc1", tag="rstdc1")

        # ---------- phase-1 emission helpers ----------
        def conv_part(xs, base, w, oh, ck, half):
            """one half (4 matmuls) of a conv chunk for out-ch oh."""
            i0, ni = base // 32, w // 32
            if half == 0:
                ck[f"ps{oh}"] = PS.tile([128, 512], f32, name="cnv",
                                        tag="util", bufs=1)
            ps = ck[f"ps{oh}"]
            ch = half
            xv = xs[ch].rearrange("p (i ki j kj) -> p ki kj i j",
                                  ki=2, kj=2, j=32)
            k = 4 * half
            for kh in range(2):
                for kw in range(2):
                    nc.tensor.matmul(
                        ps[:, 0:w],
                        w2s[ch][:, kh, kw, oh * 128:(oh + 1) * 128],
                        xv[:, kh, kw, i0:i0 + ni, :],
                        start=(k == 0), stop=(k == 7))
                    k += 1
            if half == 1:
                xr = PG.tile([128, 512], bf16, name="xr", tag=f"xr{oh}",
                             bufs=3)
                nc.vector.tensor_scalar_add(out=xr[:, 0:w], in0=ps[:, 0:w],
                                            scalar1=srb[:, oh:oh + 1])
                ck[f"xr{oh}"] = xr

        def stats_rows(ck, w):
            xrs = [ck["xr0"], ck["xr1"]]
            sq = [PG.tile([128, 512], bf16, name="sqt", tag=f"sq{ch}", bufs=2)
                  for ch in range(2)]
            for ch in range(2):
                nc.vector.tensor_mul(sq[ch][:, 0:w], xrs[ch][:, 0:w],
                                     xrs[ch][:, 0:w])
            st = PS.tile([128, 512], f32, name="st", tag="util", bufs=1)
            for ch in range(2):
                nc.tensor.matmul(st[0:1, 0:w], onescol, xrs[ch][:, 0:w],
                                 start=(ch == 0), stop=(ch == 1),
                                 tile_position=(0, 0), skip_group_check=True)
                nc.tensor.matmul(st[32:33, 0:w], onescol, sq[ch][:, 0:w],
                                 start=(ch == 0), stop=(ch == 1),
                                 tile_position=(0, 32), skip_group_check=True)
            mnneg = PG.tile([1, 512], bf16, name="mnneg", tag="mnneg", bufs=3)
            nc.vector.tensor_scalar_mul(out=mnneg[:, 0:w], in0=st[0:1, 0:w],
                                        scalar1=-1.0 / C)
            psqs = PG.tile([1, 512], f32, name="psqs", tag="psqs", bufs=2)
            nc.vector.tensor_scalar(out=psqs[:, 0:w], in0=st[32:33, 0:w],
                                    scalar1=1.0 / C, scalar2=EPS,
                                    op0=OP.mult, op1=OP.add)
            msq = PG.tile([1, 512], f32, name="msq", tag="msq", bufs=2)
            nc.vector.tensor_mul(msq[:, 0:w], mnneg[:, 0:w], mnneg[:, 0:w])
            var = PG.tile([1, 512], f32, name="var", tag="var", bufs=2)
            nc.vector.tensor_sub(var[:, 0:w], psqs[:, 0:w], msq[:, 0:w])
            ck["mnneg"] = mnneg
            ck["var"] = var

        def newton_rows(ck, w):
            """x2 (k-side): Newton rsqrt on the var ROW -> bf16 rstd row,
            broadcast via rank-1 matmul -> rbs [128, w] bf16 SBUF."""
            var = ck["var"]
            r = PG.tile([1, 512], f32, name="rw", tag="rw", bufs=2)
            nc.vector.reciprocal_approx_fast(out=r[:, 0:w], in_=var[:, 0:w])
            x = PG.tile([1, 512], f32, name="xw", tag="xw", bufs=2)
            nc.vector.tensor_scalar(out=x[:, 0:w], in0=r[:, 0:w],
                                    scalar1=0.537, scalar2=0.340,
                                    op0=OP.mult, op1=OP.add)
            s = PG.tile([1, 512], f32, name="sw", tag="sw", bufs=2)
            u = PG.tile([1, 512], f32, name="uw", tag="uw", bufs=2)
            for it in range(2):
                nc.vector.tensor_mul(s[:, 0:w], x[:, 0:w], x[:, 0:w])
                nc.vector.tensor_mul(s[:, 0:w], s[:, 0:w], var[:, 0:w])
                nc.vector.tensor_scalar(out=u[:, 0:w], in0=s[:, 0:w],
                                        scalar1=-0.5, scalar2=1.5,
                                        op0=OP.mult, op1=OP.add)
                if it == 0:
                    nc.vector.tensor_mul(x[:, 0:w], x[:, 0:w], u[:, 0:w])
            xb = PG.tile([1, 512], bf16, name="xb", tag="xb", bufs=2)
            nc.vector.tensor_mul(xb[:, 0:w], x[:, 0:w], u[:, 0:w])
            rb = PS.tile([128, 512], f32, name="rbp", tag="util", bufs=1)
            nc.tensor.matmul(rb[:, 0:w], ones1, xb[:, 0:w],
                             start=True, stop=True)
            rbs = PG.tile([128, 512], bf16, name="rbs", tag="rbs", bufs=2)
            nc.vector.tensor_copy(out=rbs[:, 0:w], in_=rb[:, 0:w])
            ck["rbs"] = rbs

        def newton_cols(ck, base, w):
            """x1 (v-side): var row -> psum columns -> Newton -> rstdc1."""
            var = ck["var"]
            nq = w // 128
            vc = PS.tile([128, 512], f32, name="vc", tag="util", bufs=1)
            for q in range(nq):
                nc.tensor.matmul(vc[:, q:q + 1],
                                 var[:, q * 128:(q + 1) * 128], one11,
                                 start=True, stop=True,
                                 skip_group_check=True)
            r = PG.tile([128, 8], f32, name="rr", tag="rr", bufs=2)
            nc.vector.reciprocal_approx_fast(out=r[:, 0:nq], in_=vc[:, 0:nq])
            x = rstdc1[:, base // 128:base // 128 + nq]
            nc.vector.tensor_scalar(out=x, in0=r[:, 0:nq],
                                    scalar1=0.537, scalar2=0.340,
                                    op0=OP.mult, op1=OP.add)
            s = PG.tile([128, 8], f32, name="ss", tag="ss", bufs=2)
            u = PG.tile([128, 8], f32, name="uu", tag="uu", bufs=2)
            for _ in range(2):
                nc.vector.tensor_mul(s[:, 0:nq], x, x)
                nc.vector.tensor_mul(s[:, 0:nq], s[:, 0:nq], vc[:, 0:nq])
                nc.vector.tensor_scalar(out=u[:, 0:nq], in0=s[:, 0:nq],
                                        scalar1=-0.5, scalar2=1.5,
                                        op0=OP.mult, op1=OP.add)
                nc.vector.tensor_mul(x, x, u[:, 0:nq])

        def k2_half(ck, base, w, g):
            ps = PS.tile([128, 512], f32, name="k2p", tag="util", bufs=1)
            for ch in range(2):
                nc.tensor.matmul(
                    ps[:, 0:w], wk[ch][:, g * 128:(g + 1) * 128],
                    ck[f"xr{ch}"][:, 0:w], start=(ch == 0), stop=False)
            nc.tensor.matmul(ps[:, 0:w], bks[:, g * 128:(g + 1) * 128],
                             ck["mnneg"][:, 0:w], start=False, stop=True)
            nc.vector.tensor_mul(k2[g][:, base:base + w], ps[:, 0:w],
                                 ck["rbs"][:, 0:w])

        def v1_one(ck, cbase, ms):
            off = ms * 128 - cbase
            ps = PS.tile([128, 512], f32, name="v1p", tag="util", bufs=1)
            for ch in range(2):
                nc.tensor.matmul(
                    ps[:, 0:C], ck[f"xr{ch}"][:, off:off + 128], wv[ch],
                    start=(ch == 0), stop=False)
            nc.tensor.matmul(ps[:, 0:C], ck["mnneg"][:, off:off + 128], bvs,
                             start=False, stop=True)
            nc.vector.tensor_scalar_mul(out=v1[ms], in0=ps[:, 0:C],
                                        scalar1=rstdc1[:, ms:ms + 1])

        def qt_chunk(n):
            for g in range(2):
                ps = PS.tile([128, 512], f32, name="qtp", tag="util", bufs=1)
                for ch in range(2):
                    nc.tensor.matmul(
                        ps, wq[ch][:, g * 128:(g + 1) * 128],
                        xq[ch][:, n * 512:(n + 1) * 512],
                        start=(ch == 0), stop=(ch == 1))
                nc.vector.tensor_copy(out=qTs[n][g], in_=ps)

        # ---- work item list (each ~<=4 matmuls) ----
        def x2_items(ci):
            base, w = X2CH[ci]
            ck = {}
            its = [lambda oh=oh, hf=hf: conv_part(x2s, base, w, oh, ck, hf)
                   for oh in range(2) for hf in range(2)]
            its.append(lambda: (stats_rows(ck, w), newton_rows(ck, w)))
            its.append(lambda: k2_half(ck, base, w, 0))
            its.append(lambda: k2_half(ck, base, w, 1))
            return its

        def x1_items(mh):
            base, w = X1CH[mh]
            ck = {}
            its = [lambda oh=oh, hf=hf: conv_part(x1s, base, w, oh, ck, hf)
                   for oh in range(2) for hf in range(2)]
            its.append(lambda: (stats_rows(ck, w), newton_cols(ck, base, w)))
            its.append(lambda: (v1_one(ck, base, base // 128),
                                v1_one(ck, base, base // 128 + 1)))
            its.append(lambda: (v1_one(ck, base, base // 128 + 2),
                                v1_one(ck, base, base // 128 + 3)))
            return its

        x1a = x1_items(0)
        x1b = x1_items(1)
        x2c2 = x2_items(2)
        work = (x2_items(1) + x1a[:4] + x2c2[:4] + x1a[4:]
                + x2c2[4:] + x1b + [lambda: qt_chunk(1)])
        widx = [0]

        def drain_one():
            if widx[0] < len(work):
                work[widx[0]]()
                widx[0] += 1

        # ---------------- prologue ----------------
        # conv first (needs only w2+x2c0+srb, first on the DMA queue);
        # qt's xq arrives during the conv burst.
        c0 = x2_items(0)
        for it in c0[:4]:
            it()
        qt_chunk(0)
        for it in c0[4:]:
            it()

        # ---------------- attention ----------------
        for n2 in range(2):
            U = [PS.tile([128, 512], f32, name=f"U{g}", tag=f"U{g}", bufs=1)
                 for g in range(2)]
            pden = PS.tile([128, 512], f32, name="pden", tag="pden", bufs=1)

            def emit_ud(E, ms, grp, pr):
                for i in range(2):
                    j = 2 * pr + i
                    h = grp * 4 + j
                    nc.tensor.matmul(
                        U[grp][32 * j:32 * j + 32, :],
                        v1[ms][:, 32 * h:32 * h + 32],
                        E[:, i * 512:(i + 1) * 512],
                        start=(ms == 0), stop=(ms == 7),
                        tile_position=(0, 32 * j),
                        skip_group_check=True)
                for i in range(2):
                    j = 2 * pr + i
                    cpos = 32 * ((j + 2) % 4)
                    nc.tensor.matmul(
                        pden[cpos:cpos + 32, :],
                        selden[:, grp, :],
                        E[:, i * 512:(i + 1) * 512],
                        start=(ms == 0 and grp == 0),
                        stop=(ms == 7 and grp == 1),
                        tile_position=(0, cpos),
                        skip_group_check=True)

            pending = []
            for blkid, (ms, grp) in enumerate(
                    (ms, grp) for ms in range(8) for grp in range(2)):
                if n2 == 0:
                    # drain all phase-1 work over the first 10 blocks, then
                    # taper the U/den lag so there is no exp-free flush tail
                    lag = 14 if blkid <= 8 else max(0, 14 - 2 * (blkid - 8))
                else:
                    lag = 4
                Ss = []
                for pr in range(2):
                    S = PS.tile([128, 1024], f32, name="S", tag="scps",
                                bufs=2)
                    for i in range(2):
                        j = 2 * pr + i
                        nc.tensor.matmul(
                            S[:, i * 512:(i + 1) * 512],
                            k2[grp][32 * j:32 * j + 32,
                                    ms * 128:(ms + 1) * 128],
                            qTs[n2][grp][32 * j:32 * j + 32, :],
                            start=True, stop=True,
                            tile_position=(32 * j, 0))
                    Ss.append(S)
                for pr in range(2):
                    E = PG.tile([128, 1024], bf16, name="E", tag="E",
                                bufs=16)
                    nc.scalar.activation(out=E, in_=Ss[pr], func=AF.Exp)
                    pending.append((E, ms, grp, pr))
                if n2 == 0:
                    drain_one()
                if n2 == 1 and blkid == 1:
                    deferred_np()
                while len(pending) > lag:
                    emit_ud(*pending.pop(0))
                if n2 == 0:
                    drain_one()
                    drain_one()
            while pending:
                emit_ud(*pending.pop(0))

            nprev = n2

            def norm_proj(U=U, pden=pden, n2=nprev):
                pdenS = PG.tile([128, 512], bf16, name="pdenS", tag="pdenS",
                                bufs=2)
                nc.vector.tensor_copy(out=pdenS, in_=pden)
                ot = []
                for g in range(2):
                    rps = PS.tile([128, 512], f32, name="rps", tag="util",
                                  bufs=1)
                    nc.tensor.matmul(rps, blk[g], pdenS, start=True, stop=True)
                    recf = PG.tile([128, 512], f32, name="recf", tag="recf",
                                   bufs=2)
                    nc.vector.reciprocal_approx_fast(out=recf, in_=rps)
                    o = PG.tile([128, 512], bf16, name="ot", tag=f"ot{g}",
                                bufs=2)
                    nc.vector.tensor_mul(o, U[g], recf)
                    ot.append(o)
                for oh in range(2):
                    psl = PS.tile([128, 512], f32, name="prj", tag="util",
                                  bufs=1)
                    for ch in range(2):
                        nc.tensor.matmul(
                            psl, wp[ch][:, oh * 128:(oh + 1) * 128], ot[ch],
                            start=(ch == 0), stop=False)
                    nc.tensor.matmul(psl, pbr[:, oh * 128:(oh + 1) * 128],
                                     ones512, start=False, stop=True)
                    y = PG.tile([128, 512], f32, name="y", tag="y", bufs=2)
                    nc.vector.tensor_copy(out=y, in_=psl)
                    nc.gpsimd.dma_start(
                        out=outt[oh * 128:(oh + 1) * 128,
                                 n2 * 512:(n2 + 1) * 512],
                        in_=y)

            if n2 == 0:
                # deferred into the start of the n2=1 loop (before its first
                # U/den pop) so the era boundary doesn't stall the exp stream
                deferred_np = norm_proj
            else:
                norm_proj()
    nc.finalize()
    return nc


def _get_program():
    if "nc" not in _prog_cache:
        _prog_cache["nc"] = _build_program()
    return _prog_cache["nc"]


def kernel(x1, x2, q_w, kv_w, sr_w, sr_b, ln_g, ln_b, proj_w, proj_b,
           H1=64, W1=64, H2=64, W2=64, **_):
    from concourse.bass_utils import run_bass_kernel_spmd

    f = np.float32
    x1 = np.asarray(x1, f)
    x2 = np.asarray(x2, f)
    q_w = np.asarray(q_w, f)
    kv_w = np.asarray(kv_w, f)
    sr_w = np.asarray(sr_w, f)
    sr_b = np.asarray(sr_b, f)
    ln_g = np.asarray(ln_g, f)
    ln_b = np.asarray(ln_b, f)
    proj_w = np.asarray(proj_w, f)
    proj_b = np.asarray(proj_b, f)

    import ml_dtypes
    bf = ml_dtypes.bfloat16
    qwT = np.ascontiguousarray(q_w.T * SCALE).astype(bf)
    kwT = np.ascontiguousarray(ln_g[:, None] * kv_w[:C].T).astype(bf)
    vwT = np.ascontiguousarray(ln_g[:, None] * kv_w[C:].T).astype(bf)
    bksum = (kv_w[:C] @ ln_g)
    bvsum = (kv_w[C:] @ ln_g)
    bvec_v = kv_w[C:] @ ln_b
    pwT = np.ascontiguousarray(proj_w.T).astype(bf)
    w2 = np.ascontiguousarray(sr_w.transpose(2, 3, 1, 0)).astype(bf)
    pbrow = np.ascontiguousarray(
        (proj_b + proj_w @ bvec_v)[None, :]).astype(bf)
    srb2 = np.ascontiguousarray(sr_b.reshape(2, 128).T).astype(f)
    blkdm = np.zeros((2, 128, 128), bf)
    for g in range(2):
        for i in range(128):
            j = i // 32
            src = 32 * ((j + 2) % 4) + g
            blkdm[g, src, i] = 1.0
    x1T = [np.ascontiguousarray(x1[b].T).astype(bf) for b in range(B)]
    x2T = [np.ascontiguousarray(x2[b].T).astype(bf) for b in range(B)]

    in_maps = []
    for core in range(8):
        b, chk = divmod(core, 4)
        in_maps.append({
            "x1t": x1T[b], "x2t": x2T[b],
            "xqt": np.ascontiguousarray(x1T[b][:, chk * NCH:(chk + 1) * NCH]),
            "w2": w2, "qwt": qwT, "kwt": kwT, "vwt": vwT, "pwt": pwT,
            "pbrow": pbrow, "srb2": srb2, "blkd": blkdm,
            "bksd": np.ascontiguousarray(bksum[None, :]).astype(bf),
            "bvsd": np.ascontiguousarray(bvsum[None, :]).astype(bf),
        })

    nc = _get_program()
    res = run_bass_kernel_spmd(nc, in_maps, core_ids=list(range(8)))
    out = np.empty((B, N, C), f)
    for core in range(8):
        b, chk = divmod(core, 4)
        out[b, chk * NCH:(chk + 1) * NCH, :] = res.results[core]["outt"].T
    return out


# revision 22
# speedup vs baseline: 1.5234x; 1.0055x over previous
"""CrossTemporalAttention2 Trainium2 kernel (v4: collective conv dedup).

Sharding: 8 cores = 2 batches x 4 query-chunks of 1024 rows. Each core
computes conv+LN+K/V for only ITS quarter of the m positions (256 of 1024)
and AllGathers the k2/v1 parts within its 4-core batch group, instead of
every core duplicating the full conv (4x less phase-1 Tensor work).

Two collectives pipeline the exchange: the k-side parts gather first so
scores can start while the v-side conv/gather still runs; U/den emission
lags the exp stream by a few pairs to cover the v1 readback.

Everything else as v3 (see kernel_v32.py): micro-interleaved emission,
LN folded into broadcast/per-partition multiplies (no ACT table switches),
4-up PE-array-tiled scores and U+den sets, full-window denominator rows,
lag-tapered U/den so the exp stream never sees a flush tail.

PSUM (8 banks): U0,U1,pden (3) + scores 2x[128,1024] (4) + util (1).
"""

import numpy as np

B, N, C = 2, 4096, 256
H, Dh = 8, 32
M = 1024
NCH = 1024
SCALE = Dh ** -0.5
EPS = 1e-5

_prog_cache = {}


def _build_program():
    import concourse.bass as bass
    import concourse.bacc as bacc
    import concourse.tile as tile
    from concourse import mybir

    f32 = mybir.dt.float32
    bf16 = mybir.dt.bfloat16
    AF = mybir.ActivationFunctionType
    OP = mybir.AluOpType

    nc = bacc.Bacc()

    x1q = nc.dram_tensor("x1q", [C, NCH], bf16, kind="ExternalInput")
    x2q = nc.dram_tensor("x2q", [C, NCH], bf16, kind="ExternalInput")
    w2d = nc.dram_tensor("w2", [2, 2, C, C], bf16, kind="ExternalInput")
    qwt = nc.dram_tensor("qwt", [C, C], bf16, kind="ExternalInput")
    kwt = nc.dram_tensor("kwt", [C, C], bf16, kind="ExternalInput")
    vwt = nc.dram_tensor("vwt", [C, C], bf16, kind="ExternalInput")
    pwt = nc.dram_tensor("pwt", [C, C], bf16, kind="ExternalInput")
    pbrow = nc.dram_tensor("pbrow", [1, C], bf16, kind="ExternalInput")
    bksd = nc.dram_tensor("bksd", [1, C], bf16, kind="ExternalInput")
    bvsd = nc.dram_tensor("bvsd", [1, C], bf16, kind="ExternalInput")
    srb2 = nc.dram_tensor("srb2", [128, 2], f32, kind="ExternalInput")
    blkd = nc.dram_tensor("blkd", [2, 128, 128], bf16, kind="ExternalInput")
    outt = nc.dram_tensor("outt", [C, NCH], f32, kind="ExternalOutput")

    RG = [[0, 1, 2, 3], [4, 5, 6, 7]]

    with nc.allow_low_precision(reason="bf16 matmul inputs; fp32 PSUM accum"), \
         tile.TileContext(nc) as tc:
      with tc.tile_pool(name="pg", bufs=1) as PG, \
           tc.tile_pool(name="psum", bufs=1, space="PSUM") as PS, \
           tc.tile_pool(name="dram", bufs=1, space="DRAM") as DR:

        w2s = [PG.tile([128, 2, 2, C], bf16, name=f"w2{ch}", tag=f"w2{ch}")
               for ch in range(2)]
        wq = [PG.tile([128, C], bf16, name=f"wq{ch}", tag=f"wq{ch}")
              for ch in range(2)]
        wk = [PG.tile([128, C], bf16, name=f"wk{ch}", tag=f"wk{ch}")
              for ch in range(2)]
        wv = [PG.tile([128, C], bf16, name=f"wv{ch}", tag=f"wv{ch}")
              for ch in range(2)]
        wp = [PG.tile([128, C], bf16, name=f"wp{ch}", tag=f"wp{ch}")
              for ch in range(2)]
        x2qs = [PG.tile([128, NCH], bf16, name=f"x2q{ch}", tag=f"x2q{ch}")
                for ch in range(2)]
        x1qs = [PG.tile([128, NCH], bf16, name=f"x1q{ch}", tag=f"x1q{ch}")
                for ch in range(2)]
        pbr = PG.tile([1, C], bf16, name="pbr", tag="pbr")
        bks = PG.tile([1, C], bf16, name="bks", tag="bks")
        bvs = PG.tile([1, C], bf16, name="bvs", tag="bvs")
        srb = PG.tile([128, 2], f32, name="srb", tag="srb")
        blk = [PG.tile([128, 128], bf16, name=f"blk{g}", tag=f"blk{g}")
               for g in range(2)]

        nc.gpsimd.dma_start(out=w2s[0], in_=w2d[:, :, 0:128, :].rearrange(
            "kh kw c o -> c kh kw o"))
        nc.gpsimd.dma_start(out=w2s[1], in_=w2d[:, :, 128:256, :].rearrange(
            "kh kw c o -> c kh kw o"))
        for ch in range(2):
            nc.gpsimd.dma_start(out=x2qs[ch],
                                in_=x2q[ch * 128:(ch + 1) * 128, :])
        nc.gpsimd.dma_start(out=srb, in_=srb2[:, :])
        for ch in range(2):
            nc.gpsimd.dma_start(out=wk[ch], in_=kwt[ch * 128:(ch + 1) * 128, :])
        nc.gpsimd.dma_start(out=bks, in_=bksd[:, :])
        for ch in range(2):
            nc.gpsimd.dma_start(out=x1qs[ch],
                                in_=x1q[ch * 128:(ch + 1) * 128, :])
        for ch in range(2):
            nc.gpsimd.dma_start(out=wv[ch], in_=vwt[ch * 128:(ch + 1) * 128, :])
        nc.gpsimd.dma_start(out=bvs, in_=bvsd[:, :])
        for ch in range(2):
            nc.gpsimd.dma_start(out=wq[ch], in_=qwt[ch * 128:(ch + 1) * 128, :])
        for ch in range(2):
            nc.gpsimd.dma_start(out=wp[ch], in_=pwt[ch * 128:(ch + 1) * 128, :])
        nc.gpsimd.dma_start(out=pbr, in_=pbrow[:, :])
        for g in range(2):
            nc.gpsimd.dma_start(out=blk[g], in_=blkd[g])

        onescol = PG.tile([128, 1], bf16, name="onescol", tag="onescol")
        nc.vector.memset(onescol, 1.0)
        one11 = PG.tile([1, 1], f32, name="one11", tag="one11")
        nc.vector.memset(one11, 1.0)
        ones1 = PG.tile([1, 128], bf16, name="ones1", tag="ones1")
        nc.vector.memset(ones1, 1.0)
        ones512 = PG.tile([1, 512], bf16, name="ones512", tag="ones512")
        nc.vector.memset(ones512, 1.0)
        selden = PG.tile([128, 2, 32], bf16, name="selden", tag="selden")
        nc.vector.memset(selden, 0.0)
        for g in range(2):
            nc.vector.memset(selden[:, g, g:g + 1], 1.0)

        k2 = [PG.tile([128, M], bf16, name=f"k2{g}", tag=f"k2{g}")
              for g in range(2)]
        v1 = [PG.tile([128, C], bf16, name=f"v1_{ms}", tag=f"v1_{ms}")
              for ms in range(8)]
        k2own = [PG.tile([128, 256], bf16, name=f"k2o{g}", tag=f"k2o{g}")
                 for g in range(2)]
        v1own = [PG.tile([128, 256], bf16, name=f"v1o{a}", tag=f"v1o{a}")
                 for a in range(2)]
        qTs = [[PG.tile([128, 512], bf16, name=f"qT{n}{g}", tag=f"qT{n}{g}")
                for g in range(2)] for n in range(2)]
        rstdc1 = PG.tile([128, 8], f32, name="rstdc1", tag="rstdc1")

        kparts = DR.tile([2, 128, 256], bf16, name="kparts")
        kgath = DR.tile([4, 2, 128, 256], bf16, name="kgath")
        vparts = DR.tile([2, 128, 256], bf16, name="vparts")
        vgath = DR.tile([4, 2, 128, 256], bf16, name="vgath")

        # ---------- phase-1 (local quarter only) ----------
        def conv_quarter(xs, ck):
            for oh in range(2):
                ps = PS.tile([128, 512], f32, name="cnv", tag="util", bufs=1)
                k = 0
                for ch in range(2):
                    xv = xs[ch].rearrange("p (i ki j kj) -> p ki kj i j",
                                          ki=2, kj=2, j=32)
                    for kh in range(2):
                        for kw in range(2):
                            nc.tensor.matmul(
                                ps[:, 0:256],
                                w2s[ch][:, kh, kw, oh * 128:(oh + 1) * 128],
                                xv[:, kh, kw, 0:8, :],
                                start=(k == 0), stop=(k == 7))
                            k += 1
                xr = PG.tile([128, 256], bf16, name="xr", tag=f"xr{oh}",
                             bufs=2)
                nc.vector.tensor_scalar_add(out=xr, in0=ps[:, 0:256],
                                            scalar1=srb[:, oh:oh + 1])
                ck[f"xr{oh}"] = xr

        def stats_rows(ck):
            w = 256
            xrs = [ck["xr0"], ck["xr1"]]
            sq = [PG.tile([128, 256], bf16, name="sqt", tag=f"sq{ch}", bufs=2)
                  for ch in range(2)]
            for ch in range(2):
                nc.vector.tensor_mul(sq[ch], xrs[ch], xrs[ch])
            st = PS.tile([128, 512], f32, name="st", tag="util", bufs=1)
            for ch in range(2):
                nc.tensor.matmul(st[0:1, 0:w], onescol, xrs[ch],
                                 start=(ch == 0), stop=(ch == 1),
                                 tile_position=(0, 0), skip_group_check=True)
                nc.tensor.matmul(st[32:33, 0:w], onescol, sq[ch],
                                 start=(ch == 0), stop=(ch == 1),
                                 tile_position=(0, 32), skip_group_check=True)
            mnneg = PG.tile([1, 256], bf16, name="mnneg", tag="mnneg", bufs=2)
            nc.vector.tensor_scalar_mul(out=mnneg, in0=st[0:1, 0:w],
                                        scalar1=-1.0 / C)
            psqs = PG.tile([1, 256], f32, name="psqs", tag="psqs", bufs=2)
            nc.vector.tensor_scalar(out=psqs, in0=st[32:33, 0:w],
                                    scalar1=1.0 / C, scalar2=EPS,
                                    op0=OP.mult, op1=OP.add)
            msq = PG.tile([1, 256], f32, name="msq", tag="msq", bufs=2)
            nc.vector.tensor_mul(msq, mnneg, mnneg)
            var = PG.tile([1, 256], f32, name="var", tag="var", bufs=2)
            nc.vector.tensor_sub(var, psqs, msq)
            ck["mnneg"] = mnneg
            ck["var"] = var

        def newton_rows(ck):
            w = 256
            var = ck["var"]
            r = PG.tile([1, 256], f32, name="rw", tag="rw", bufs=2)
            nc.vector.reciprocal_approx_fast(out=r, in_=var)
            x = PG.tile([1, 256], f32, name="xw", tag="xw", bufs=2)
            nc.vector.tensor_scalar(out=x, in0=r, scalar1=0.537,
                                    scalar2=0.340, op0=OP.mult, op1=OP.add)
            s = PG.tile([1, 256], f32, name="sw", tag="sw", bufs=2)
            u = PG.tile([1, 256], f32, name="uw", tag="uw", bufs=2)
            for it in range(2):
                nc.vector.tensor_mul(s, x, x)
                nc.vector.tensor_mul(s, s, var)
                nc.vector.tensor_scalar(out=u, in0=s, scalar1=-0.5,
                                        scalar2=1.5, op0=OP.mult, op1=OP.add)
                if it == 0:
                    nc.vector.tensor_mul(x, x, u)
            xb = PG.tile([1, 256], bf16, name="xb", tag="xb", bufs=2)
            nc.vector.tensor_mul(xb, x, u)
            rb = PS.tile([128, 512], f32, name="rbp", tag="util", bufs=1)
            nc.tensor.matmul(rb[:, 0:w], ones1, xb, start=True, stop=True)
            rbs = PG.tile([128, 256], bf16, name="rbs", tag="rbs", bufs=2)
            nc.vector.tensor_copy(out=rbs, in_=rb[:, 0:w])
            ck["rbs"] = rbs

        def newton_cols(ck):
            var = ck["var"]
            vc = PS.tile([128, 512], f32, name="vc", tag="util", bufs=1)
            for q in range(2):
                nc.tensor.matmul(vc[:, q:q + 1],
                                 var[:, q * 128:(q + 1) * 128], one11,
                                 start=True, stop=True,
                                 skip_group_check=True)
            r = PG.tile([128, 8], f32, name="rr", tag="rr", bufs=2)
            nc.vector.reciprocal_approx_fast(out=r[:, 0:2], in_=vc[:, 0:2])
            x = rstdc1[:, 0:2]
            nc.vector.tensor_scalar(out=x, in0=r[:, 0:2], scalar1=0.537,
                                    scalar2=0.340, op0=OP.mult, op1=OP.add)
            s = PG.tile([128, 8], f32, name="ss", tag="ss", bufs=2)
            u = PG.tile([128, 8], f32, name="uu", tag="uu", bufs=2)
            for _ in range(2):
                nc.vector.tensor_mul(s[:, 0:2], x, x)
                nc.vector.tensor_mul(s[:, 0:2], s[:, 0:2], vc[:, 0:2])
                nc.vector.tensor_scalar(out=u[:, 0:2], in0=s[:, 0:2],
                                        scalar1=-0.5, scalar2=1.5,
                                        op0=OP.mult, op1=OP.add)
                nc.vector.tensor_mul(x, x, u[:, 0:2])

        def qt_chunk(n):
            for g in range(2):
                ps = PS.tile([128, 512], f32, name="qtp", tag="util", bufs=1)
                for ch in range(2):
                    nc.tensor.matmul(
                        ps, wq[ch][:, g * 128:(g + 1) * 128],
                        x1qs[ch][:, n * 512:(n + 1) * 512],
                        start=(ch == 0), stop=(ch == 1))
                nc.vector.tensor_copy(out=qTs[n][g], in_=ps)

        # ---- k side: conv, stats, k2 parts, gather ----
        ck2 = {}
        conv_quarter(x2qs, ck2)
        stats_rows(ck2)
        newton_rows(ck2)
        for g in range(2):
            ps = PS.tile([128, 512], f32, name="k2p", tag="util", bufs=1)
            for ch in range(2):
                nc.tensor.matmul(
                    ps[:, 0:256], wk[ch][:, g * 128:(g + 1) * 128],
                    ck2[f"xr{ch}"], start=(ch == 0), stop=False)
            nc.tensor.matmul(ps[:, 0:256], bks[:, g * 128:(g + 1) * 128],
                             ck2["mnneg"], start=False, stop=True)
            nc.vector.tensor_mul(k2own[g], ps[:, 0:256], ck2["rbs"])
            nc.gpsimd.dma_start(out=kparts[g], in_=k2own[g])
        nc.gpsimd.collective_compute(
            "AllGather", OP.bypass, replica_groups=RG,
            ins=[kparts.opt()], outs=[kgath.opt()])

        # ---- v side: conv, stats, v1 parts, gather ----
        ck1 = {}
        conv_quarter(x1qs, ck1)
        stats_rows(ck1)
        newton_cols(ck1)
        for a in range(2):
            ps = PS.tile([128, 512], f32, name="v1p", tag="util", bufs=1)
            for ch in range(2):
                nc.tensor.matmul(
                    ps[:, 0:C], ck1[f"xr{ch}"][:, a * 128:(a + 1) * 128],
                    wv[ch], start=(ch == 0), stop=False)
            nc.tensor.matmul(ps[:, 0:C], ck1["mnneg"][:, a * 128:(a + 1) * 128],
                             bvs, start=False, stop=True)
            nc.vector.tensor_scalar_mul(out=v1own[a], in0=ps[:, 0:C],
                                        scalar1=rstdc1[:, a:a + 1])
            nc.gpsimd.dma_start(out=vparts[a], in_=v1own[a])
        nc.gpsimd.collective_compute(
            "AllGather", OP.bypass, replica_groups=RG,
            ins=[vparts.opt()], outs=[vgath.opt()])

        # q projections run while the collectives are in flight
        qt_chunk(0)
        qt_chunk(1)

        # ---- readback ----
        for g in range(2):
            nc.gpsimd.dma_start(
                out=k2[g].rearrange("p (s c) -> p s c", s=4),
                in_=kgath[:, g, :, :].rearrange("s p c -> p s c"))
        for s in range(4):
            for a in range(2):
                nc.gpsimd.dma_start(out=v1[2 * s + a], in_=vgath[s, a, :, :])

        # ---------------- attention ----------------
        deferred_np = None
        for n2 in range(2):
            U = [PS.tile([128, 512], f32, name=f"U{g}", tag=f"U{g}", bufs=1)
                 for g in range(2)]
            pden = PS.tile([128, 512], f32, name="pden", tag="pden", bufs=1)

            def emit_ud(E, ms, grp, pr):
                for i in range(2):
                    j = 2 * pr + i
                    h = grp * 4 + j
                    nc.tensor.matmul(
                        U[grp][32 * j:32 * j + 32, :],
                        v1[ms][:, 32 * h:32 * h + 32],
                        E[:, i * 512:(i + 1) * 512],
                        start=(ms == 0), stop=(ms == 7),
                        tile_position=(0, 32 * j),
                        skip_group_check=True)
                for i in range(2):
                    j = 2 * pr + i
                    cpos = 32 * ((j + 2) % 4)
                    nc.tensor.matmul(
                        pden[cpos:cpos + 32, :],
                        selden[:, grp, :],
                        E[:, i * 512:(i + 1) * 512],
                        start=(ms == 0 and grp == 0),
                        stop=(ms == 7 and grp == 1),
                        tile_position=(0, cpos),
                        skip_group_check=True)

            pending = []
            for blkid, (ms, grp) in enumerate(
                    (ms, grp) for ms in range(8) for grp in range(2)):
                lag = 6 if (n2 == 0 and blkid < 12) else \
                    (max(0, 6 - 2 * (blkid - 11)) if n2 == 0 else 4)
                Ss = []
                for pr in range(2):
                    S = PS.tile([128, 1024], f32, name="S", tag="scps",
                                bufs=2)
                    for i in range(2):
                        j = 2 * pr + i
                        nc.tensor.matmul(
                            S[:, i * 512:(i + 1) * 512],
                            k2[grp][32 * j:32 * j + 32,
                                    ms * 128:(ms + 1) * 128],
                            qTs[n2][grp][32 * j:32 * j + 32, :],
                            start=True, stop=True,
                            tile_position=(32 * j, 0))
                    Ss.append(S)
                for pr in range(2):
                    E = PG.tile([128, 1024], bf16, name="E", tag="E",
                                bufs=16)
                    nc.scalar.activation(out=E, in_=Ss[pr], func=AF.Exp)
                    pending.append((E, ms, grp, pr))
                if n2 == 1 and blkid == 1:
                    deferred_np()
                while len(pending) > lag:
                    emit_ud(*pending.pop(0))
            while pending:
                emit_ud(*pending.pop(0))

            nprev = n2

            def norm_proj(U=U, pden=pden, n2=nprev):
                pdenS = PG.tile([128, 512], bf16, name="pdenS", tag="pdenS",
                                bufs=2)
                nc.vector.tensor_copy(out=pdenS, in_=pden)
                ot = []
                for g in range(2):
                    rps = PS.tile([128, 512], f32, name="rps", tag="util",
                                  bufs=1)
                    nc.tensor.matmul(rps, blk[g], pdenS, start=True, stop=True)
                    recf = PG.tile([128, 512], f32, name="recf", tag="recf",
                                   bufs=2)
                    nc.vector.reciprocal_approx_fast(out=recf, in_=rps)
                    o = PG.tile([128, 512], bf16, name="ot", tag=f"ot{g}",
                                bufs=2)
                    nc.vector.tensor_mul(o, U[g], recf)
                    ot.append(o)
                for oh in range(2):
                    psl = PS.tile([128, 512], f32, name="prj", tag="util",
                                  bufs=1)
                    for ch in range(2):
                        nc.tensor.matmul(
                            psl, wp[ch][:, oh * 128:(oh + 1) * 128], ot[ch],
                            start=(ch == 0), stop=False)
                    nc.tensor.matmul(psl, pbr[:, oh * 128:(oh + 1) * 128],
                                     ones512, start=False, stop=True)
                    y = PG.tile([128, 512], f32, name="y", tag="y", bufs=2)
                    nc.vector.tensor_copy(out=y, in_=psl)
                    nc.gpsimd.dma_start(
                        out=outt[oh * 128:(oh + 1) * 128,
                                 n2 * 512:(n2 + 1) * 512],
                        in_=y)

            if n2 == 0:
                deferred_np = norm_proj
            else:
                norm_proj()
    nc.finalize()
    return nc


def _get_program():
    if "nc" not in _prog_cache:
        _prog_cache["nc"] = _build_program()
    return _prog_cache["nc"]


def kernel(x1, x2, q_w, kv_w, sr_w, sr_b, ln_g, ln_b, proj_w, proj_b,
           H1=64, W1=64, H2=64, W2=64, **_):
    from concourse.bass_utils import run_bass_kernel_spmd

    f = np.float32
    x1 = np.asarray(x1, f)
    x2 = np.asarray(x2, f)
    q_w = np.asarray(q_w, f)
    kv_w = np.asarray(kv_w, f)
    sr_w = np.asarray(sr_w, f)
    sr_b = np.asarray(sr_b, f)
    ln_g = np.asarray(ln_g, f)
    ln_b = np.asarray(ln_b, f)
    proj_w = np.asarray(proj_w, f)
    proj_b = np.asarray(proj_b, f)

    import ml_dtypes
    bf = ml_dtypes.bfloat16
    qwT = np.ascontiguousarray(q_w.T * SCALE).astype(bf)
    kwT = np.ascontiguousarray(ln_g[:, None] * kv_w[:C].T).astype(bf)
    vwT = np.ascontiguousarray(ln_g[:, None] * kv_w[C:].T).astype(bf)
    bksum = (kv_w[:C] @ ln_g)
    bvsum = (kv_w[C:] @ ln_g)
    bvec_v = kv_w[C:] @ ln_b
    pwT = np.ascontiguousarray(proj_w.T).astype(bf)
    w2 = np.ascontiguousarray(sr_w.transpose(2, 3, 1, 0)).astype(bf)
    pbrow = np.ascontiguousarray(
        (proj_b + proj_w @ bvec_v)[None, :]).astype(bf)
    srb2 = np.ascontiguousarray(sr_b.reshape(2, 128).T).astype(f)
    blkdm = np.zeros((2, 128, 128), bf)
    for g in range(2):
        for i in range(128):
            j = i // 32
            src = 32 * ((j + 2) % 4) + g
            blkdm[g, src, i] = 1.0
    x1T = [np.ascontiguousarray(x1[b].T).astype(bf) for b in range(B)]
    x2T = [np.ascontiguousarray(x2[b].T).astype(bf) for b in range(B)]

    in_maps = []
    for core in range(8):
        b, chk = divmod(core, 4)
        sl = slice(chk * NCH, (chk + 1) * NCH)
        in_maps.append({
            "x1q": np.ascontiguousarray(x1T[b][:, sl]),
            "x2q": np.ascontiguousarray(x2T[b][:, sl]),
            "w2": w2, "qwt": qwT, "kwt": kwT, "vwt": vwT, "pwt": pwT,
            "pbrow": pbrow, "srb2": srb2, "blkd": blkdm,
            "bksd": np.ascontiguousarray(bksum[None, :]).astype(bf),
            "bvsd": np.ascontiguousarray(bvsum[None, :]).astype(bf),
        })

    nc = _get_program()
    res = run_bass_kernel_spmd(nc, in_maps, core_ids=list(range(8)))
    out = np.empty((B, N, C), f)
    for core in range(8):
        b, chk = divmod(core, 4)
        out[b, chk * NCH:(chk + 1) * NCH, :] = res.results[core]["outt"].T
    return out
